# revision 10
# baseline (speedup 1.0000x reference)
"""Trainium2 Bass kernel for nn_MultiHeadAttention_678604832830.

Strategy: data-parallel over batch B=8 -> one batch element per NeuronCore.
Each core computes, for its batch element b:
  - per-head attention probs  attn[h, i, j]  (written to HBM, f32)
  - ln = LayerNorm(attn_out @ proj_w.T + proj_b + q)   (written to HBM, f32)
Host reassembles attn_flat / one_head_attn from the per-core outputs.

Matmuls run as float32r (tf32-class precision at bf16 throughput); the
AV contraction runs in bf16 (attention probs are O(1), error ~1e-3).
Scores are computed twice: natural [i,j] for the attn output + softmax
sums (mask folded into one fused DVE tensor_tensor_reduce), and
transposed [j,i] for the AV contraction (mask added inside PSUM by an
identity (x) (-1e9*mask^T) matmul so exp() zeroes masked lanes free).
"""

import sys

sys.path.insert(0, "/opt/trn_rl_repo")

import numpy as np
import ml_dtypes

import concourse.bass as bass
import concourse.mybir as mybir
import concourse.tile as tile
from concourse.bass_utils import run_bass_kernel_spmd
from concourse.vector_clock import ScopedClock

F32 = mybir.dt.float32
F32R = mybir.dt.float32r
BF16 = mybir.dt.bfloat16
AF = mybir.ActivationFunctionType
ALU = mybir.AluOpType

D, H, DK, B, L = 1024, 16, 64, 8, 1024
CC = 8   # chunks of 128 over the concat-head dim (H*DK = 1024)
DC = 8   # chunks of 128 over d_model
JC = 8   # chunks of 128 over key positions
IH = 2   # halves of the query dim (512 each)
IT = 4   # 128-row tiles per query half
SCALE = 1.0 / 32.0  # 1/sqrt(d_model)
NEG = -1.0e9
LN_EPS = 1e-6


def _patch_tile():
    """Lift the stale 192KB/partition SBUF cap to the usable 208KB."""
    if getattr(tile.TileContext, "_mha_patched", False):
        return
    try:
        from concourse import tile_utils

        if getattr(tile_utils, "max_sbuf_usage", 0) < 208 * 1024:
            tile_utils.max_sbuf_usage = 208 * 1024
    except Exception:
        pass
    tile.TileContext._mha_patched = True


def _legalize_waits(nc):
    """This walrus build accepts at most ONE sync wait per instruction (two
    for EventSemaphore). Tile sometimes attaches more (self-loading f32r
    matmuls have no LDWEIGHTS to spill onto; the final drain gets the whole
    global-clock set). Move excess waits onto same-engine nops inserted
    immediately before the offending instruction — the engine's in-order
    queue preserves the wait-before-execute semantics."""
    import bass_rust as _br

    n_split = 0
    for f in nc.m.functions:
        for bb in f.blocks:
            new = []
            changed = False
            for inst in bb.instructions:
                si = inst.sync_info
                cap = 2 if isinstance(inst, _br.InstEventSemaphore) else 1
                if si is not None and len(si.on_wait) > cap:
                    waits = list(si.on_wait)
                    SyncInfo = type(si)
                    for w in waits[:-cap]:
                        nop = _br.InstNoOp(
                            name=f"{inst.name}-lw{n_split}",
                            engine=inst.engine,
                            ins=[],
                            outs=[],
                        )
                        n_split += 1
                        nop.sync_info = SyncInfo(on_wait=[w], on_update=[])
                        nc.register_instruction(nop)
                        new.append(nop)
                    inst.sync_info = SyncInfo(
                        on_wait=waits[-cap:], on_update=list(si.on_update)
                    )
                    changed = True
                new.append(inst)
            if changed:
                bb.instructions = new


def _build_nc():
    _patch_tile()
    nc = bass.Bass()

    # ---- DRAM tensors (per-core; same NEFF on all 8 cores) ----
    qT = nc.dram_tensor("qT", [D, L], F32R, kind="ExternalInput")
    kT = nc.dram_tensor("kT", [D, L], F32R, kind="ExternalInput")
    vT = nc.dram_tensor("vT", [D, L], F32R, kind="ExternalInput")
    wq = nc.dram_tensor("wq", [D, H * DK], F32R, kind="ExternalInput")
    wk = nc.dram_tensor("wk", [D, H * DK], F32R, kind="ExternalInput")
    wv = nc.dram_tensor("wv", [D, H * DK], F32R, kind="ExternalInput")
    pwT = nc.dram_tensor("pwT", [H * DK, D], F32R, kind="ExternalInput")
    qres = nc.dram_tensor("qres", [L, D], F32, kind="ExternalInput")
    mbT = nc.dram_tensor("mbT", [L, L], BF16, kind="ExternalInput")   # -1e9*mask^T [j,i]
    m01 = nc.dram_tensor("m01", [L, L], BF16, kind="ExternalInput")   # 1-mask [i,j]
    gb = nc.dram_tensor("gb", [2, 128, D], F32, kind="ExternalInput")  # ln_g/ln_b bcast
    id_bf = nc.dram_tensor("id_bf", [128, 128], BF16, kind="ExternalInput")
    id_fr = nc.dram_tensor("id_fr", [128, 128], F32R, kind="ExternalInput")
    attn = nc.dram_tensor("attn", [H, L, L], F32, kind="ExternalOutput")
    ln = nc.dram_tensor("ln", [L, D], F32, kind="ExternalOutput")

    with tile.TileContext(nc) as tc:
        from contextlib import ExitStack

        with ExitStack() as ctx:
            # ---------- persistent tiles ----------
            persist = ctx.enter_context(tc.tile_pool(name="persist", bufs=1))
            kt_sb = persist.tile([128, CC, L], F32R)      # K~^T  [c_in, cc, j]   4MiB
            qt_sb = persist.tile([128, CC, L], F32R)      # Q~^T  [c_in, cc, i]   4MiB
            v_sb = persist.tile([128, JC, H * DK], BF16)  # V~    [j_in, jc, call] 2MiB
            pw_sb = persist.tile([128, CC, D], F32R)      # proj_w^T [call_in, cc, d] 4MiB
            mbT_sb = persist.tile([128, JC, 512], BF16)   # mask bias^T half  1MiB
            m01_sb = persist.tile([128, IT, L], BF16)     # 0/1 mask rows half 1MiB
            id_bf_sb = persist.tile([128, 128], BF16)
            id_fr_sb = persist.tile([128, 128], F32R)
            g_sb = persist.tile([128, D], F32)
            b_sb = persist.tile([128, D], F32)
            eps_sb = persist.tile([128, 1], F32)
            nc.vector.memset(eps_sb, LN_EPS)
            nc.sync.dma_start(id_bf_sb, id_bf.ap())
            nc.sync.dma_start(id_fr_sb, id_fr.ap())
            nc.sync.dma_start(g_sb, gb.ap()[0])
            nc.sync.dma_start(b_sb, gb.ap()[1])
            nc.sync.dma_start(
                pw_sb, pwT.ap().rearrange("(cc p) d -> p cc d", p=128)
            )

            # ---------- phase P0: Q~^T, K~^T, V~ projections ----------
            with tc.tile_pool(name="p0in", bufs=1) as p0in, \
                 tc.tile_pool(name="p0ps", bufs=2, space="PSUM") as p0ps:
                for xdram, wdram, kind in (
                    (kT, wk, "k"), (qT, wq, "q"), (vT, wv, "v"),
                ):
                    xin = p0in.tile([128, DC, L], F32R, tag="xin")
                    win = p0in.tile([128, DC, H * DK], F32R, tag="win")
                    nc.sync.dma_start(
                        xin, xdram.ap().rearrange("(dc p) l -> p dc l", p=128)
                    )
                    nc.sync.dma_start(
                        win, wdram.ap().rearrange("(dc p) c -> p dc c", p=128)
                    )
                    if kind in ("k", "q"):
                        dst = kt_sb if kind == "k" else qt_sb
                        for cc in range(CC):
                            for lh in range(2):
                                ps = p0ps.tile([128, 512], F32, tag="p0")
                                for dc in range(DC):
                                    nc.tensor.matmul(
                                        ps,
                                        win[:, dc, cc * 128 : (cc + 1) * 128],
                                        xin[:, dc, lh * 512 : (lh + 1) * 512],
                                        start=(dc == 0),
                                        stop=(dc == DC - 1),
                                    )
                                nc.any.tensor_copy(
                                    out=dst[:, cc, lh * 512 : (lh + 1) * 512],
                                    in_=ps,
                                )
                    else:
                        for lc in range(JC):
                            for ch in range(2):
                                ps = p0ps.tile([128, 512], F32, tag="p0")
                                for dc in range(DC):
                                    nc.tensor.matmul(
                                        ps,
                                        xin[:, dc, lc * 128 : (lc + 1) * 128],
                                        win[:, dc, ch * 512 : (ch + 1) * 512],
                                        start=(dc == 0),
                                        stop=(dc == DC - 1),
                                    )
                                nc.any.tensor_copy(
                                    out=v_sb[:, lc, ch * 512 : (ch + 1) * 512],
                                    in_=ps,
                                )

            # ---------- PSUM pools (8 banks total) ----------
            psB = ctx.enter_context(tc.tile_pool(name="psB", bufs=2, space="PSUM"))
            psA = ctx.enter_context(tc.tile_pool(name="psA", bufs=2, space="PSUM"))
            psAV = ctx.enter_context(tc.tile_pool(name="psAV", bufs=2, space="PSUM"))
            psT = ctx.enter_context(tc.tile_pool(name="psT", bufs=1, space="PSUM"))
            psO = ctx.enter_context(tc.tile_pool(name="psO", bufs=1, space="PSUM"))

            # ---------- working pools ----------
            eT_pool = ctx.enter_context(tc.tile_pool(name="eT", bufs=2))
            ework = ctx.enter_context(tc.tile_pool(name="ework", bufs=2))
            stats = ctx.enter_context(tc.tile_pool(name="stats", bufs=10))
            onat_pool = ctx.enter_context(tc.tile_pool(name="onat", bufs=1))
            otT_pool = ctx.enter_context(tc.tile_pool(name="otT", bufs=9))
            lnw = ctx.enter_context(tc.tile_pool(name="lnw", bufs=2))

            for ih in range(IH):
                i0 = ih * 512

                # masks for this query half
                nc.sync.dma_start(
                    mbT_sb,
                    mbT.ap()[:, i0 : i0 + 512].rearrange(
                        "(jc p) i -> p jc i", p=128
                    ),
                )
                nc.sync.dma_start(
                    m01_sb,
                    m01.ap()[i0 : i0 + 512, :].rearrange(
                        "(it p) j -> p it j", p=128
                    ),
                )

                onat = onat_pool.tile([128, IT, H * DK], F32R, tag="onat")

                # ---- P2: heads ----
                for h in range(H):
                    cc_h = h // 2
                    cr = 64 * (h % 2)
                    inv_s_tiles = []

                    # pass A: natural scores -> attn rows + softmax sums
                    for it in range(IT):
                        ia = i0 + it * 128
                        e_raw = ework.tile([128, L], F32, tag="e_raw")
                        for jh in range(2):
                            ps = psA.tile([128, 512], F32, tag="psA")
                            nc.tensor.matmul(
                                ps,
                                qt_sb[
                                    cr : cr + 64, cc_h, ia : ia + 128
                                ],
                                kt_sb[
                                    cr : cr + 64, cc_h, jh * 512 : (jh + 1) * 512
                                ],
                                start=True,
                                stop=True,
                            )
                            nc.scalar.activation(
                                out=e_raw[:, jh * 512 : (jh + 1) * 512],
                                in_=ps,
                                func=AF.Exp,
                                scale=SCALE,
                            )
                        e_m = ework.tile([128, L], F32, tag="e_m")
                        s_sum = stats.tile([128, 1], F32, tag="s")
                        nc.vector.scalar_tensor_tensor(
                            out=e_m,
                            in0=e_raw,
                            scalar=1.0,
                            in1=m01_sb[:, it, :],
                            op0=ALU.mult,
                            op1=ALU.mult,
                            accum_out=s_sum,
                        )
                        inv_s = stats.tile([128, 1], F32, tag="inv")
                        nc.vector.reciprocal(out=inv_s, in_=s_sum)
                        inv_s_tiles.append(inv_s)
                        nc.vector.tensor_scalar_mul(
                            out=e_m, in0=e_m, scalar1=inv_s
                        )
                        nc.sync.dma_start(attn.ap()[h, ia : ia + 128, :], e_m)

                    # pass B: transposed scores (+mask in PSUM) -> e^T (bf16)
                    eT = eT_pool.tile([128, JC, 512], BF16, tag="eT")
                    for jc in range(JC):
                        ps = psB.tile([128, 512], F32, tag="psB")
                        nc.tensor.matmul(
                            ps,
                            kt_sb[cr : cr + 64, cc_h, jc * 128 : (jc + 1) * 128],
                            qt_sb[cr : cr + 64, cc_h, i0 : i0 + 512],
                            start=True,
                            stop=False,
                        )
                        nc.tensor.matmul(
                            ps,
                            id_bf_sb,
                            mbT_sb[:, jc, :],
                            start=False,
                            stop=True,
                        )
                        nc.scalar.activation(
                            out=eT[:, jc, :], in_=ps, func=AF.Exp, scale=SCALE
                        )

                    # AV: out[i, c] = sum_j eT[j,i] * V[j,c], normalized on evac
                    for it in range(IT):
                        ps = psAV.tile([128, 64], F32, tag="psAV")
                        for jc in range(JC):
                            nc.tensor.matmul(
                                ps,
                                eT[:, jc, it * 128 : (it + 1) * 128],
                                v_sb[:, jc, h * DK : (h + 1) * DK],
                                start=(jc == 0),
                                stop=(jc == JC - 1),
                            )
                        nc.vector.tensor_scalar_mul(
                            out=onat[:, it, h * DK : (h + 1) * DK],
                            in0=ps,
                            scalar1=inv_s_tiles[it],
                        )

                # ---- P3/P4: transpose out, project, residual, LayerNorm ----
                for it in range(IT):
                    ia = i0 + it * 128
                    oT_tiles = []
                    for cc in range(CC):
                        pst = psT.tile([128, 128], F32R, tag="psT")
                        nc.tensor.transpose(
                            pst,
                            onat[:, it, cc * 128 : (cc + 1) * 128],
                            id_fr_sb,
                        )
                        oT = otT_pool.tile([128, 128], F32R, tag="oT")
                        nc.any.tensor_copy(out=oT, in_=pst)
                        oT_tiles.append(oT)
                    res = lnw.tile([128, D], F32, tag="res")
                    nc.sync.dma_start(res, qres.ap()[ia : ia + 128, :])
                    for dh in range(2):
                        ps = psO.tile([128, 512], F32, tag="psO")
                        for cc in range(CC):
                            nc.tensor.matmul(
                                ps,
                                oT_tiles[cc],
                                pw_sb[:, cc, dh * 512 : (dh + 1) * 512],
                                start=(cc == 0),
                                stop=(cc == CC - 1),
                            )
                        nc.vector.tensor_add(
                            out=res[:, dh * 512 : (dh + 1) * 512],
                            in0=ps,
                            in1=res[:, dh * 512 : (dh + 1) * 512],
                        )
                    # LayerNorm over d
                    st = lnw.tile([128, 2, nc.vector.BN_STATS_DIM], F32, tag="bnst")
                    for g in range(2):
                        nc.vector.bn_stats(
                            out=st[:, g, :], in_=res[:, g * 512 : (g + 1) * 512]
                        )
                    mv = lnw.tile([128, nc.vector.BN_AGGR_DIM], F32, tag="bnag")
                    nc.vector.bn_aggr(out=mv, in_=st)
                    rstd = lnw.tile([128, 1], F32, tag="rstd")
                    nc.scalar.activation(
                        out=rstd, in_=mv[:, 1:2], func=AF.Sqrt, bias=eps_sb,
                        scale=1.0,
                    )
                    nc.vector.reciprocal(out=rstd, in_=rstd)
                    nc.vector.tensor_scalar(
                        out=res,
                        in0=res,
                        scalar1=mv[:, 0:1],
                        scalar2=rstd,
                        op0=ALU.subtract,
                        op1=ALU.mult,
                    )
                    nc.vector.tensor_mul(out=res, in0=res, in1=g_sb)
                    nc.vector.tensor_add(out=res, in0=res, in1=b_sb)
                    nc.sync.dma_start(ln.ap()[ia : ia + 128, :], res)

    _legalize_waits(nc)
    return nc


_NC = None


def _get_nc():
    global _NC
    if _NC is None:
        _NC = _build_nc()
    return _NC


def _prep_in_maps(inputs):
    q = np.asarray(inputs["q"], np.float32)
    k = np.asarray(inputs["k"], np.float32)
    v = np.asarray(inputs["v"], np.float32)
    mask = np.asarray(inputs["attn_mask"])
    w_q = np.asarray(inputs["w_q"], np.float32)
    w_k = np.asarray(inputs["w_k"], np.float32)
    w_v = np.asarray(inputs["w_v"], np.float32)
    proj_w = np.asarray(inputs["proj_w"], np.float32)
    proj_b = np.asarray(inputs["proj_b"], np.float32)
    ln_g = np.asarray(inputs["ln_g"], np.float32)
    ln_b = np.asarray(inputs["ln_b"], np.float32)

    wq_cat = np.ascontiguousarray(w_q.transpose(1, 0, 2).reshape(D, H * DK))
    wk_cat = np.ascontiguousarray(w_k.transpose(1, 0, 2).reshape(D, H * DK))
    wv_cat = np.ascontiguousarray(w_v.transpose(1, 0, 2).reshape(D, H * DK))
    pwT_h = np.ascontiguousarray(proj_w.T)
    gb_h = np.stack(
        [np.broadcast_to(ln_g, (128, D)), np.broadcast_to(ln_b, (128, D))]
    ).astype(np.float32)
    id_bf_h = np.eye(128, dtype=ml_dtypes.bfloat16)
    id_fr_h = np.eye(128, dtype=np.float32)

    in_maps = []
    for b in range(B):
        m_b = mask[b]
        in_maps.append(
            {
                "qT": np.ascontiguousarray(q[b].T),
                "kT": np.ascontiguousarray(k[b].T),
                "vT": np.ascontiguousarray(v[b].T),
                "wq": wq_cat,
                "wk": wk_cat,
                "wv": wv_cat,
                "pwT": pwT_h,
                "qres": np.ascontiguousarray(q[b] + proj_b),
                "mbT": np.ascontiguousarray(
                    (NEG * m_b.T.astype(np.float32)).astype(ml_dtypes.bfloat16)
                ),
                "m01": np.ascontiguousarray(
                    (1.0 - m_b.astype(np.float32)).astype(ml_dtypes.bfloat16)
                ),
                "gb": gb_h,
                "id_bf": id_bf_h,
                "id_fr": id_fr_h,
            }
        )
    return in_maps


def kernel(**inputs):
    in_maps = _prep_in_maps(inputs)
    nc = _get_nc()
    res = run_bass_kernel_spmd(nc, in_maps, core_ids=list(range(B)))

    attn_all = np.stack([res.results[b]["attn"] for b in range(B)])  # [B, H, L, L]
    ln_all = np.stack([res.results[b]["ln"] for b in range(B)])      # [B, L, D]
    attn_flat = np.ascontiguousarray(
        attn_all.transpose(1, 0, 2, 3).reshape(H * B, L, L)
    )
    one_head = np.ascontiguousarray(attn_all[0, ::2])  # attn[2b, batch 0]
    return ln_all, attn_flat, one_head


# revision 32
# speedup vs baseline: 47.1279x; 47.1279x over previous
"""Trainium2 Bass kernel for nn_MultiHeadAttention_678604832830.

Strategy: data-parallel over batch B=8 -> one batch element per NeuronCore.
Each core computes, for its batch element b:
  - per-head attention probs  attn[h, i, j]  (written to HBM, f32)
  - ln = LayerNorm(attn_out @ proj_w.T + proj_b + q)   (written to HBM, f32)
Host reassembles attn_flat / one_head_attn from the per-core outputs.

Matmuls run as float32r (tf32-class precision at bf16 throughput); the
AV contraction runs in bf16 (attention probs are O(1), error ~1e-3).
Scores are computed twice: natural [i,j] for the attn output + softmax
sums (mask folded into one fused DVE scalar_tensor_tensor), and
transposed [j,i] for the AV contraction (mask added inside PSUM by an
identity (x) (-1e9*mask^T) matmul so exp() zeroes masked lanes free).

Projection phase runs V -> K -> Q so the head loop (which needs Q last)
starts as early as possible; all phases share two PSUM pools to avoid
bank-reuse serialization barriers.
"""

import sys

sys.path.insert(0, "/opt/trn_rl_repo")

import numpy as np
import ml_dtypes

import concourse.bass as bass
import concourse.mybir as mybir
import concourse.tile as tile
from concourse.bass_utils import run_bass_kernel_spmd

F32 = mybir.dt.float32
F32R = mybir.dt.float32r
BF16 = mybir.dt.bfloat16
AF = mybir.ActivationFunctionType
ALU = mybir.AluOpType

D, H, DK, B, L = 1024, 16, 64, 8, 1024
CC = 8   # chunks of 128 over the concat-head dim (H*DK = 1024)
DC = 8   # chunks of 128 over d_model
JC = 8   # chunks of 128 over key positions
IH = 2   # halves of the query dim (512 each)
IT = 4   # 128-row tiles per query half
SCALE = 1.0 / 32.0  # 1/sqrt(d_model)
NEG = -1.0e9
LN_EPS = 1e-6


def _patch_tile():
    """Lift the stale 192KB/partition SBUF cap to the usable 208KB."""
    if getattr(tile.TileContext, "_mha_patched", False):
        return
    try:
        from concourse import tile_utils

        if getattr(tile_utils, "max_sbuf_usage", 0) < 208 * 1024:
            tile_utils.max_sbuf_usage = 208 * 1024
    except Exception:
        pass
    tile.TileContext._mha_patched = True


def _legalize_waits(nc):
    """This walrus build accepts at most ONE sync wait per instruction (two
    for EventSemaphore). Tile sometimes attaches more (self-loading f32r
    matmuls have no LDWEIGHTS to spill onto; the final drain gets the whole
    global-clock set). Move excess waits onto same-engine nops inserted
    immediately before the offending instruction — the engine's in-order
    queue preserves the wait-before-execute semantics."""
    import bass_rust as _br

    n_split = 0
    for f in nc.m.functions:
        for bb in f.blocks:
            new = []
            changed = False
            for inst in bb.instructions:
                si = inst.sync_info
                cap = 2 if isinstance(inst, _br.InstEventSemaphore) else 1
                if si is not None and len(si.on_wait) > cap:
                    waits = list(si.on_wait)
                    SyncInfo = type(si)
                    for w in waits[:-cap]:
                        nop = _br.InstNoOp(
                            name=f"{inst.name}-lw{n_split}",
                            engine=inst.engine,
                            ins=[],
                            outs=[],
                        )
                        n_split += 1
                        nop.sync_info = SyncInfo(on_wait=[w], on_update=[])
                        nc.register_instruction(nop)
                        new.append(nop)
                    inst.sync_info = SyncInfo(
                        on_wait=waits[-cap:], on_update=list(si.on_update)
                    )
                    changed = True
                new.append(inst)
            if changed:
                bb.instructions = new


def _build_nc():
    _patch_tile()
    nc = bass.Bass()

    # ---- DRAM tensors (per-core; same NEFF on all 8 cores) ----
    qT = nc.dram_tensor("qT", [D, L], BF16, kind="ExternalInput")
    kT = nc.dram_tensor("kT", [D, L], BF16, kind="ExternalInput")
    vT = nc.dram_tensor("vT", [D, L], BF16, kind="ExternalInput")
    wq = nc.dram_tensor("wq", [D, H * DK], BF16, kind="ExternalInput")
    wk = nc.dram_tensor("wk", [D, H * DK], BF16, kind="ExternalInput")
    wv = nc.dram_tensor("wv", [D, H * DK], BF16, kind="ExternalInput")
    pwT = nc.dram_tensor("pwT", [H * DK, D], F32R, kind="ExternalInput")
    qres = nc.dram_tensor("qres", [L, D], F32, kind="ExternalInput")
    mbT = nc.dram_tensor("mbT", [L, L], BF16, kind="ExternalInput")   # -1e9*mask^T [j,i]
    m01 = nc.dram_tensor("m01", [L, L], BF16, kind="ExternalInput")   # 1-mask [i,j]
    gb = nc.dram_tensor("gb", [2, 128, D], BF16, kind="ExternalInput")  # ln_g/ln_b bcast
    id_bf = nc.dram_tensor("id_bf", [128, 128], BF16, kind="ExternalInput")
    id_fr = nc.dram_tensor("id_fr", [128, 128], F32R, kind="ExternalInput")
    attn = nc.dram_tensor("attn", [H, L, L], F32, kind="ExternalOutput")
    ln = nc.dram_tensor("ln", [L, D], F32, kind="ExternalOutput")

    with tile.TileContext(nc) as tc:
        from contextlib import ExitStack

        with ExitStack() as ctx:
            # ---------- persistent tiles (~13.1 MiB) ----------
            persist = ctx.enter_context(tc.tile_pool(name="persist", bufs=1))
            kt_sb = persist.tile([128, CC, L], F32R)      # K~^T  [c_in, cc, j]   4MiB
            qt_sb = persist.tile([128, CC, L], F32R)      # Q~^T  [c_in, cc, i]   4MiB
            v_sb = persist.tile([128, JC, H * DK], BF16)  # V~    [j_in, jc, call] 2MiB
            mbT_sb = persist.tile([128, JC, 512], BF16)   # mask bias^T half  1MiB
            m01_sb = persist.tile([128, IT, L], BF16)     # 0/1 mask rows half 1MiB
            id_bf_sb = persist.tile([128, 128], BF16)
            id_fr_sb = persist.tile([128, 128], F32R)
            g_sb = persist.tile([128, D], BF16)
            b_sb = persist.tile([128, D], BF16)
            eps_sb = persist.tile([128, 1], F32)
            nc.vector.memset(eps_sb, LN_EPS)
            nc.sync.dma_start(id_bf_sb, id_bf.ap())
            nc.sync.dma_start(id_fr_sb, id_fr.ap())
            nc.sync.dma_start(g_sb, gb.ap()[0])
            nc.sync.dma_start(b_sb, gb.ap()[1])

            # ---------- PSUM pools: 3x[128,1024] scores + 2x[128,512] ----
            psP = ctx.enter_context(tc.tile_pool(name="psP", bufs=3, space="PSUM"))
            psS = ctx.enter_context(tc.tile_pool(name="psS", bufs=2, space="PSUM"))

            eT_pool = ctx.enter_context(tc.tile_pool(name="eT", bufs=2))
            ework = ctx.enter_context(tc.tile_pool(name="ework", bufs=2))
            stats = ctx.enter_context(tc.tile_pool(name="stats", bufs=10))
            onat_pool = ctx.enter_context(tc.tile_pool(name="onat", bufs=1))

            # ---------- phase P0 part 1: V~ ----------
            # ih=0 masks load first so heads can start during P0.
            def load_masks(ih):
                i0 = ih * 512
                nc.sync.dma_start(
                    mbT_sb,
                    mbT.ap()[:, i0 : i0 + 512].rearrange(
                        "(jc p) i -> p jc i", p=128
                    ),
                )
                nc.sync.dma_start(
                    m01_sb,
                    m01.ap()[i0 : i0 + 512, :].rearrange(
                        "(it p) j -> p it j", p=128
                    ),
                )

            load_masks(0)

            p0ctx = ExitStack()
            p0in = p0ctx.enter_context(tc.tile_pool(name="p0in", bufs=2))
            p0w = p0ctx.enter_context(tc.tile_pool(name="p0w", bufs=2))
            xins = {}
            for kind, xdram in (("v", vT), ("k", kT), ("q", qT)):
                xin = p0in.tile([128, DC, L], BF16, tag="xin", name=f"xin_{kind}")
                xr = xdram.ap().rearrange("(dc p) l -> p dc l", p=128)
                for dc in range(DC):
                    nc.sync.dma_start(xin[:, dc, :], xr[:, dc, :])
                xins[kind] = xin
                if kind == "v":
                    wr = wv.ap().rearrange("(dc p) c -> p dc c", p=128)
                    for quarter in range(4):
                        c0 = quarter * 256
                        win = p0w.tile([128, DC, 256], BF16, tag="winq",
                                       name="win_v")
                        for dc in range(DC):
                            nc.sync.dma_start(
                                win[:, dc, :], wr[:, dc, c0 : c0 + 256]
                            )
                        for lc in range(JC):
                            ps_full = psP.tile(
                                [128, L], F32, tag="ps", name="p0psv"
                            )
                            ps = ps_full[:, :256]
                            for dc in range(DC):
                                nc.tensor.matmul(
                                    ps,
                                    xin[:, dc, lc * 128 : (lc + 1) * 128],
                                    win[:, dc, :],
                                    start=(dc == 0),
                                    stop=(dc == DC - 1),
                                )
                            nc.vector.tensor_copy(
                                out=v_sb[:, lc, c0 : c0 + 256], in_=ps
                            )

            def emit_kq_quarter(quarter):
                c0 = quarter * 256
                for kind, wdram in (("k", wk), ("q", wq)):
                    dst = kt_sb if kind == "k" else qt_sb
                    xin = xins[kind]
                    wr = wdram.ap().rearrange("(dc p) c -> p dc c", p=128)
                    win = p0w.tile([128, DC, 256], BF16, tag="winq",
                                   name=f"win_{kind}{quarter}")
                    for dc in range(DC):
                        nc.sync.dma_start(
                            win[:, dc, :], wr[:, dc, c0 : c0 + 256]
                        )
                    for ccl in range(2):
                        cc = quarter * 2 + ccl
                        ps = psP.tile([128, L], F32, tag="ps", name="p0ps")
                        for lh in range(2):
                            for dc in range(DC):
                                nc.tensor.matmul(
                                    ps[:, lh * 512 : (lh + 1) * 512],
                                    win[:, dc, ccl * 128 : (ccl + 1) * 128],
                                    xin[:, dc, lh * 512 : (lh + 1) * 512],
                                    start=(dc == 0),
                                    stop=(dc == DC - 1),
                                )
                        nc.vector.tensor_copy(out=dst[:, cc, :], in_=ps)

            def emit_head(ih, h, onat):
                i0 = ih * 512
                cc_h = h // 2
                cr = 64 * (h % 2)
                inv_s_tiles = []
                # pass A: natural scores -> attn rows + softmax sums
                for it in range(IT):
                    ia = i0 + it * 128
                    e_raw = ework.tile([128, L], F32, tag="e_raw")
                    ps = psP.tile([128, L], F32, tag="ps", name="ps_a")
                    for jh in range(2):
                        nc.tensor.matmul(
                            ps[:, jh * 512 : (jh + 1) * 512],
                            qt_sb[cr : cr + 64, cc_h, ia : ia + 128],
                            kt_sb[
                                cr : cr + 64, cc_h, jh * 512 : (jh + 1) * 512
                            ],
                            start=True,
                            stop=True,
                        )
                    nc.scalar.activation(
                        out=e_raw, in_=ps, func=AF.Exp, scale=SCALE
                    )
                    e_m = ework.tile([128, L], F32, tag="e_m")
                    s_sum = stats.tile([128, 1], F32, tag="s")
                    nc.vector.scalar_tensor_tensor(
                        out=e_m,
                        in0=e_raw,
                        scalar=1.0,
                        in1=m01_sb[:, it, :],
                        op0=ALU.mult,
                        op1=ALU.mult,
                        accum_out=s_sum,
                    )
                    inv_s = stats.tile([128, 1], F32, tag="inv")
                    nc.vector.reciprocal(out=inv_s, in_=s_sum)
                    inv_s_tiles.append(inv_s)
                    nc.vector.tensor_scalar_mul(out=e_m, in0=e_m, scalar1=inv_s)
                    nc.sync.dma_start(attn.ap()[h, ia : ia + 128, :], e_m)

                # pass B: transposed scores (+mask in PSUM) -> e^T (bf16)
                eT = eT_pool.tile([128, JC, 512], BF16, tag="eT")
                for jp in range(JC // 2):
                    ps = psP.tile([128, L], F32, tag="ps", name="ps_b")
                    for sub in range(2):
                        jc = jp * 2 + sub
                        nc.tensor.matmul(
                            ps[:, sub * 512 : (sub + 1) * 512],
                            kt_sb[cr : cr + 64, cc_h, jc * 128 : (jc + 1) * 128],
                            qt_sb[cr : cr + 64, cc_h, i0 : i0 + 512],
                            start=True,
                            stop=False,
                        )
                        nc.tensor.matmul(
                            ps[:, sub * 512 : (sub + 1) * 512],
                            id_bf_sb,
                            mbT_sb[:, jc, :],
                            start=False,
                            stop=True,
                        )
                    nc.scalar.activation(
                        out=eT[:, jp * 2 : jp * 2 + 2, :],
                        in_=ps,
                        func=AF.Exp,
                        scale=SCALE,
                    )

                # AV: out[i, c] = sum_j eT[j,i] * V[j,c], normalized on evac
                for it in range(IT):
                    ps_full = psS.tile([128, 512], F32, tag="pss", name="ps_av")
                    ps = ps_full[:, :64]
                    for jc in range(JC):
                        nc.tensor.matmul(
                            ps,
                            eT[:, jc, it * 128 : (it + 1) * 128],
                            v_sb[:, jc, h * DK : (h + 1) * DK],
                            start=(jc == 0),
                            stop=(jc == JC - 1),
                        )
                    nc.vector.tensor_scalar_mul(
                        out=onat[:, it, h * DK : (h + 1) * DK],
                        in0=ps,
                        scalar1=inv_s_tiles[it],
                    )

            def emit_p3(ih, onat):
                for it in range(IT):
                    ia = ih * 512 + it * 128
                    oT_tiles = []
                    for cc in range(CC):
                        pst_full = psS.tile(
                            [128, 512], F32R, tag="pss", name="ps_tr"
                        )
                        pst = pst_full[:, :128]
                        nc.tensor.transpose(
                            pst,
                            onat[:, it, cc * 128 : (cc + 1) * 128],
                            id_fr_sb,
                        )
                        oT = otT_pool.tile([128, 128], F32R, tag="oT")
                        nc.any.tensor_copy(out=oT, in_=pst)
                        oT_tiles.append(oT)
                    res = lnw.tile([128, D], F32, tag="res")
                    nc.sync.dma_start(res, qres.ap()[ia : ia + 128, :])
                    for dh in range(2):
                        ps_o = psS.tile([128, 512], F32, tag="pss", name="ps_o")
                        for cc in range(CC):
                            nc.tensor.matmul(
                                ps_o,
                                oT_tiles[cc],
                                pw_sb[:, cc, dh * 512 : (dh + 1) * 512],
                                start=(cc == 0),
                                stop=(cc == CC - 1),
                            )
                        nc.vector.tensor_add(
                            out=res[:, dh * 512 : (dh + 1) * 512],
                            in0=ps_o,
                            in1=res[:, dh * 512 : (dh + 1) * 512],
                        )
                    st = lnw.tile([128, 2, nc.vector.BN_STATS_DIM], F32, tag="bnst")
                    for g in range(2):
                        nc.vector.bn_stats(
                            out=st[:, g, :], in_=res[:, g * 512 : (g + 1) * 512]
                        )
                    mv = lnw.tile([128, nc.vector.BN_AGGR_DIM], F32, tag="bnag")
                    nc.vector.bn_aggr(out=mv, in_=st)
                    rstd = lnw.tile([128, 1], F32, tag="rstd")
                    nc.scalar.activation(
                        out=rstd, in_=mv[:, 1:2], func=AF.Sqrt, bias=eps_sb,
                        scale=1.0,
                    )
                    nc.vector.reciprocal(out=rstd, in_=rstd)
                    nc.vector.tensor_scalar(
                        out=res,
                        in0=res,
                        scalar1=mv[:, 0:1],
                        scalar2=rstd,
                        op0=ALU.subtract,
                        op1=ALU.mult,
                    )
                    nc.gpsimd.tensor_mul(out=res, in0=res, in1=g_sb)
                    nc.gpsimd.tensor_add(out=res, in0=res, in1=b_sb)
                    nc.sync.dma_start(ln.ap()[ia : ia + 128, :], res)

            # ---- pipelined emission: quarters unblock head groups (ih=0) ----
            onat0 = onat_pool.tile([128, IT, H * DK], F32R, tag="onat", name="onat0")
            for quarter in range(4):
                emit_kq_quarter(quarter)
                for h in range(4 * quarter, 4 * quarter + 4):
                    emit_head(0, h, onat0)
            p0ctx.close()
            otT_pool = ctx.enter_context(tc.tile_pool(name="otT", bufs=8))
            lnw = ctx.enter_context(tc.tile_pool(name="lnw", bufs=2))
            pw_pool = ctx.enter_context(tc.tile_pool(name="pw", bufs=1))
            pw_sb = pw_pool.tile([128, CC, D], F32R)  # proj_w^T  4MiB
            nc.sync.dma_start(
                pw_sb, pwT.ap().rearrange("(cc p) d -> p cc d", p=128)
            )
            emit_p3(0, onat0)
            load_masks(1)
            onat1 = onat_pool.tile([128, IT, H * DK], F32R, tag="onat", name="onat1")
            for h in range(H):
                emit_head(1, h, onat1)
            emit_p3(1, onat1)

    _legalize_waits(nc)
    return nc


_NC = None


def _get_nc():
    global _NC
    if _NC is None:
        _NC = _build_nc()
    return _NC


def _prep_in_maps(inputs):
    q = np.asarray(inputs["q"], np.float32)
    k = np.asarray(inputs["k"], np.float32)
    v = np.asarray(inputs["v"], np.float32)
    mask = np.asarray(inputs["attn_mask"])
    w_q = np.asarray(inputs["w_q"], np.float32)
    w_k = np.asarray(inputs["w_k"], np.float32)
    w_v = np.asarray(inputs["w_v"], np.float32)
    proj_w = np.asarray(inputs["proj_w"], np.float32)
    proj_b = np.asarray(inputs["proj_b"], np.float32)
    ln_g = np.asarray(inputs["ln_g"], np.float32)
    ln_b = np.asarray(inputs["ln_b"], np.float32)

    wq_cat = np.ascontiguousarray(w_q.transpose(1, 0, 2).reshape(D, H * DK))
    wk_cat = np.ascontiguousarray(w_k.transpose(1, 0, 2).reshape(D, H * DK))
    wv_cat = np.ascontiguousarray(w_v.transpose(1, 0, 2).reshape(D, H * DK))
    pwT_h = np.ascontiguousarray(proj_w.T)
    gb_h = np.stack(
        [np.broadcast_to(ln_g, (128, D)), np.broadcast_to(ln_b, (128, D))]
    ).astype(ml_dtypes.bfloat16)
    id_bf_h = np.eye(128, dtype=ml_dtypes.bfloat16)
    id_fr_h = np.eye(128, dtype=np.float32)

    in_maps = []
    for b in range(B):
        m_b = mask[b]
        in_maps.append(
            {
                "qT": np.ascontiguousarray(q[b].T).astype(ml_dtypes.bfloat16),
                "kT": np.ascontiguousarray(k[b].T).astype(ml_dtypes.bfloat16),
                "vT": np.ascontiguousarray(v[b].T).astype(ml_dtypes.bfloat16),
                "wq": wq_cat.astype(ml_dtypes.bfloat16),
                "wk": wk_cat.astype(ml_dtypes.bfloat16),
                "wv": wv_cat.astype(ml_dtypes.bfloat16),
                "pwT": pwT_h,
                "qres": np.ascontiguousarray(q[b] + proj_b),
                "mbT": np.ascontiguousarray(
                    (NEG * m_b.T.astype(np.float32)).astype(ml_dtypes.bfloat16)
                ),
                "m01": np.ascontiguousarray(
                    (1.0 - m_b.astype(np.float32)).astype(ml_dtypes.bfloat16)
                ),
                "gb": gb_h,
                "id_bf": id_bf_h,
                "id_fr": id_fr_h,
            }
        )
    return in_maps


def kernel(**inputs):
    in_maps = _prep_in_maps(inputs)
    nc = _get_nc()
    res = run_bass_kernel_spmd(nc, in_maps, core_ids=list(range(B)))

    attn_all = np.stack([res.results[b]["attn"] for b in range(B)])  # [B, H, L, L]
    ln_all = np.stack([res.results[b]["ln"] for b in range(B)])      # [B, L, D]
    attn_flat = np.ascontiguousarray(
        attn_all.transpose(1, 0, 2, 3).reshape(H * B, L, L)
    )
    one_head = np.ascontiguousarray(attn_all[0, ::2])  # attn[2b, batch 0]
    return ln_all, attn_flat, one_head


# revision 38
# speedup vs baseline: 47.1702x; 1.0009x over previous
"""Trainium2 Bass kernel for nn_MultiHeadAttention_678604832830.

Strategy: data-parallel over batch B=8 -> one batch element per NeuronCore.
Each core computes, for its batch element b:
  - per-head attention probs  attn[h, i, j]  (written to HBM, f32)
  - ln = LayerNorm(attn_out @ proj_w.T + proj_b + q)   (written to HBM, f32)
Host reassembles attn_flat / one_head_attn from the per-core outputs.

Matmuls run as float32r (tf32-class precision at bf16 throughput); the
AV contraction runs in bf16 (attention probs are O(1), error ~1e-3).
Scores are computed twice: natural [i,j] for the attn output + softmax
sums (mask folded into one fused DVE scalar_tensor_tensor), and
transposed [j,i] for the AV contraction (mask added inside PSUM by an
identity (x) (-1e9*mask^T) matmul so exp() zeroes masked lanes free).

Projection phase runs V -> K -> Q so the head loop (which needs Q last)
starts as early as possible; all phases share two PSUM pools to avoid
bank-reuse serialization barriers.
"""

import sys

sys.path.insert(0, "/opt/trn_rl_repo")

import numpy as np
import ml_dtypes

import concourse.bass as bass
import concourse.mybir as mybir
import concourse.tile as tile
from concourse.bass_utils import run_bass_kernel_spmd

F32 = mybir.dt.float32
F32R = mybir.dt.float32r
BF16 = mybir.dt.bfloat16
AF = mybir.ActivationFunctionType
ALU = mybir.AluOpType

D, H, DK, B, L = 1024, 16, 64, 8, 1024
CC = 8   # chunks of 128 over the concat-head dim (H*DK = 1024)
DC = 8   # chunks of 128 over d_model
JC = 8   # chunks of 128 over key positions
IH = 2   # halves of the query dim (512 each)
IT = 4   # 128-row tiles per query half
SCALE = 1.0 / 32.0  # 1/sqrt(d_model)
NEG = -1.0e9
LN_EPS = 1e-6


def _patch_tile():
    """Lift the stale 192KB/partition SBUF cap to the usable 208KB."""
    if getattr(tile.TileContext, "_mha_patched", False):
        return
    try:
        from concourse import tile_utils

        if getattr(tile_utils, "max_sbuf_usage", 0) < 208 * 1024:
            tile_utils.max_sbuf_usage = 208 * 1024
    except Exception:
        pass
    tile.TileContext._mha_patched = True


def _legalize_waits(nc):
    """This walrus build accepts at most ONE sync wait per instruction (two
    for EventSemaphore). Tile sometimes attaches more (self-loading f32r
    matmuls have no LDWEIGHTS to spill onto; the final drain gets the whole
    global-clock set). Move excess waits onto same-engine nops inserted
    immediately before the offending instruction — the engine's in-order
    queue preserves the wait-before-execute semantics."""
    import bass_rust as _br

    n_split = 0
    for f in nc.m.functions:
        for bb in f.blocks:
            new = []
            changed = False
            for inst in bb.instructions:
                si = inst.sync_info
                cap = 2 if isinstance(inst, _br.InstEventSemaphore) else 1
                if si is not None and len(si.on_wait) > cap:
                    waits = list(si.on_wait)
                    SyncInfo = type(si)
                    for w in waits[:-cap]:
                        nop = _br.InstNoOp(
                            name=f"{inst.name}-lw{n_split}",
                            engine=inst.engine,
                            ins=[],
                            outs=[],
                        )
                        n_split += 1
                        nop.sync_info = SyncInfo(on_wait=[w], on_update=[])
                        nc.register_instruction(nop)
                        new.append(nop)
                    inst.sync_info = SyncInfo(
                        on_wait=waits[-cap:], on_update=list(si.on_update)
                    )
                    changed = True
                new.append(inst)
            if changed:
                bb.instructions = new


def _build_nc():
    _patch_tile()
    nc = bass.Bass()

    # ---- DRAM tensors (per-core; same NEFF on all 8 cores) ----
    qT = nc.dram_tensor("qT", [D, L], BF16, kind="ExternalInput")
    kT = nc.dram_tensor("kT", [D, L], BF16, kind="ExternalInput")
    vT = nc.dram_tensor("vT", [D, L], BF16, kind="ExternalInput")
    wq = nc.dram_tensor("wq", [D, H * DK], BF16, kind="ExternalInput")
    wk = nc.dram_tensor("wk", [D, H * DK], BF16, kind="ExternalInput")
    wv = nc.dram_tensor("wv", [D, H * DK], BF16, kind="ExternalInput")
    pwT = nc.dram_tensor("pwT", [H * DK, D], F32R, kind="ExternalInput")
    qres = nc.dram_tensor("qres", [L, D], F32, kind="ExternalInput")
    mbT = nc.dram_tensor("mbT", [L, L], BF16, kind="ExternalInput")   # -1e9*mask^T [j,i]
    m01 = nc.dram_tensor("m01", [L, L], BF16, kind="ExternalInput")   # 1-mask [i,j]
    gb = nc.dram_tensor("gb", [2, 128, D], BF16, kind="ExternalInput")  # ln_g/ln_b bcast
    id_bf = nc.dram_tensor("id_bf", [128, 128], BF16, kind="ExternalInput")
    id_fr = nc.dram_tensor("id_fr", [128, 128], F32R, kind="ExternalInput")
    attn = nc.dram_tensor("attn", [H, L, L], F32, kind="ExternalOutput")
    ln = nc.dram_tensor("ln", [L, D], F32, kind="ExternalOutput")

    with tile.TileContext(nc) as tc:
        from contextlib import ExitStack

        with ExitStack() as ctx:
            # ---------- persistent tiles (~13.1 MiB) ----------
            persist = ctx.enter_context(tc.tile_pool(name="persist", bufs=1))
            kt_sb = persist.tile([128, CC, L], F32R)      # K~^T  [c_in, cc, j]   4MiB
            qt_sb = persist.tile([128, CC, L], F32R)      # Q~^T  [c_in, cc, i]   4MiB
            v_sb = persist.tile([128, JC, H * DK], BF16)  # V~    [j_in, jc, call] 2MiB
            mbT_sb = persist.tile([128, JC, 512], BF16)   # mask bias^T half  1MiB
            m01_sb = persist.tile([128, IT, L], BF16)     # 0/1 mask rows half 1MiB
            id_bf_sb = persist.tile([128, 128], BF16)
            id_fr_sb = persist.tile([128, 128], F32R)
            g_sb = persist.tile([128, D], BF16)
            b_sb = persist.tile([128, D], BF16)
            eps_sb = persist.tile([128, 1], F32)
            nc.vector.memset(eps_sb, LN_EPS)
            nc.sync.dma_start(id_bf_sb, id_bf.ap())
            nc.sync.dma_start(id_fr_sb, id_fr.ap())
            nc.sync.dma_start(g_sb, gb.ap()[0])
            nc.sync.dma_start(b_sb, gb.ap()[1])

            # ---------- PSUM pools: 3x[128,1024] scores + 2x[128,512] ----
            psP = ctx.enter_context(tc.tile_pool(name="psP", bufs=3, space="PSUM"))
            psS = ctx.enter_context(tc.tile_pool(name="psS", bufs=2, space="PSUM"))

            eT_pool = ctx.enter_context(tc.tile_pool(name="eT", bufs=2))
            ework = ctx.enter_context(tc.tile_pool(name="ework", bufs=2))
            stats = ctx.enter_context(tc.tile_pool(name="stats", bufs=10))
            onat_pool = ctx.enter_context(tc.tile_pool(name="onat", bufs=1))
            otT_pool = ctx.enter_context(tc.tile_pool(name="otT", bufs=9))

            # ---------- phase P0 part 1: V~ ----------
            # ih=0 masks load first so heads can start during P0.
            def load_masks(ih):
                i0 = ih * 512
                nc.sync.dma_start(
                    mbT_sb,
                    mbT.ap()[:, i0 : i0 + 512].rearrange(
                        "(jc p) i -> p jc i", p=128
                    ),
                )
                nc.sync.dma_start(
                    m01_sb,
                    m01.ap()[i0 : i0 + 512, :].rearrange(
                        "(it p) j -> p it j", p=128
                    ),
                )

            load_masks(0)

            p0ctx = ExitStack()
            p0in = p0ctx.enter_context(tc.tile_pool(name="p0in", bufs=2))
            p0w = p0ctx.enter_context(tc.tile_pool(name="p0w", bufs=2))
            xins = {}

            def load_xin(kind, xdram):
                xin = p0in.tile([128, DC, L], BF16, tag="xin", name=f"xin_{kind}")
                xr = xdram.ap().rearrange("(dc p) l -> p dc l", p=128)
                for dc in range(DC):
                    nc.sync.dma_start(xin[:, dc, :], xr[:, dc, :])
                xins[kind] = xin

            def emit_v_phase():
                xin = xins["v"]
                wr = wv.ap().rearrange("(dc p) c -> p dc c", p=128)
                for quarter in range(4):
                    c0 = quarter * 256
                    win = p0w.tile([128, DC, 256], BF16, tag="winq",
                                   name="win_v")
                    for dc in range(DC):
                        nc.sync.dma_start(
                            win[:, dc, :], wr[:, dc, c0 : c0 + 256]
                        )
                    for lc in range(JC):
                        ps_full = psP.tile(
                            [128, L], F32, tag="ps", name="p0psv"
                        )
                        ps = ps_full[:, :256]
                        for dc in range(DC):
                            nc.tensor.matmul(
                                ps,
                                xin[:, dc, lc * 128 : (lc + 1) * 128],
                                win[:, dc, :],
                                start=(dc == 0),
                                stop=(dc == DC - 1),
                            )
                        nc.vector.tensor_copy(
                            out=v_sb[:, lc, c0 : c0 + 256], in_=ps
                        )

            def emit_kq_quarter(quarter):
                c0 = quarter * 256
                for kind, wdram in (("k", wk), ("q", wq)):
                    dst = kt_sb if kind == "k" else qt_sb
                    xin = xins[kind]
                    wr = wdram.ap().rearrange("(dc p) c -> p dc c", p=128)
                    win = p0w.tile([128, DC, 256], BF16, tag="winq",
                                   name=f"win_{kind}{quarter}")
                    for dc in range(DC):
                        nc.sync.dma_start(
                            win[:, dc, :], wr[:, dc, c0 : c0 + 256]
                        )
                    for ccl in range(2):
                        cc = quarter * 2 + ccl
                        ps = psP.tile([128, L], F32, tag="ps", name="p0ps")
                        for lh in range(2):
                            for dc in range(DC):
                                nc.tensor.matmul(
                                    ps[:, lh * 512 : (lh + 1) * 512],
                                    win[:, dc, ccl * 128 : (ccl + 1) * 128],
                                    xin[:, dc, lh * 512 : (lh + 1) * 512],
                                    start=(dc == 0),
                                    stop=(dc == DC - 1),
                                )
                        nc.vector.tensor_copy(out=dst[:, cc, :], in_=ps)

            def emit_head(ih, h, onat):
                i0 = ih * 512
                cc_h = h // 2
                cr = 64 * (h % 2)
                inv_s_tiles = []
                # pass A: natural scores -> attn rows + softmax sums
                for it in range(IT):
                    ia = i0 + it * 128
                    e_raw = ework.tile([128, L], F32, tag="e_raw")
                    ps = psP.tile([128, L], F32, tag="ps", name="ps_a")
                    for jh in range(2):
                        nc.tensor.matmul(
                            ps[:, jh * 512 : (jh + 1) * 512],
                            qt_sb[cr : cr + 64, cc_h, ia : ia + 128],
                            kt_sb[
                                cr : cr + 64, cc_h, jh * 512 : (jh + 1) * 512
                            ],
                            start=True,
                            stop=True,
                        )
                    nc.scalar.activation(
                        out=e_raw, in_=ps, func=AF.Exp, scale=SCALE
                    )
                    e_m = ework.tile([128, L], F32, tag="e_m")
                    s_sum = stats.tile([128, 1], F32, tag="s")
                    nc.vector.scalar_tensor_tensor(
                        out=e_m,
                        in0=e_raw,
                        scalar=1.0,
                        in1=m01_sb[:, it, :],
                        op0=ALU.mult,
                        op1=ALU.mult,
                        accum_out=s_sum,
                    )
                    inv_s = stats.tile([128, 1], F32, tag="inv")
                    nc.vector.reciprocal(out=inv_s, in_=s_sum)
                    inv_s_tiles.append(inv_s)
                    nc.vector.tensor_scalar_mul(out=e_m, in0=e_m, scalar1=inv_s)
                    nc.sync.dma_start(attn.ap()[h, ia : ia + 128, :], e_m)

                # pass B: transposed scores (+mask in PSUM) -> e^T (bf16)
                eT = eT_pool.tile([128, JC, 512], BF16, tag="eT")
                for jp in range(JC // 2):
                    ps = psP.tile([128, L], F32, tag="ps", name="ps_b")
                    for sub in range(2):
                        jc = jp * 2 + sub
                        nc.tensor.matmul(
                            ps[:, sub * 512 : (sub + 1) * 512],
                            kt_sb[cr : cr + 64, cc_h, jc * 128 : (jc + 1) * 128],
                            qt_sb[cr : cr + 64, cc_h, i0 : i0 + 512],
                            start=True,
                            stop=False,
                        )
                        nc.tensor.matmul(
                            ps[:, sub * 512 : (sub + 1) * 512],
                            id_bf_sb,
                            mbT_sb[:, jc, :],
                            start=False,
                            stop=True,
                        )
                    nc.scalar.activation(
                        out=eT[:, jp * 2 : jp * 2 + 2, :],
                        in_=ps,
                        func=AF.Exp,
                        scale=SCALE,
                    )

                # AV: out[i, c] = sum_j eT[j,i] * V[j,c], normalized on evac
                for it in range(IT):
                    ps_full = psS.tile([128, 512], F32, tag="pss", name="ps_av")
                    ps = ps_full[:, :64]
                    for jc in range(JC):
                        nc.tensor.matmul(
                            ps,
                            eT[:, jc, it * 128 : (it + 1) * 128],
                            v_sb[:, jc, h * DK : (h + 1) * DK],
                            start=(jc == 0),
                            stop=(jc == JC - 1),
                        )
                    nc.vector.tensor_scalar_mul(
                        out=onat[:, it, h * DK : (h + 1) * DK],
                        in0=ps,
                        scalar1=inv_s_tiles[it],
                    )

            def emit_transposes(ih, it, onat, oT_store):
                for cc in range(CC):
                    pst_full = psS.tile(
                        [128, 512], F32R, tag="pss", name="ps_tr"
                    )
                    pst = pst_full[:, :128]
                    nc.tensor.transpose(
                        pst,
                        onat[:, it, cc * 128 : (cc + 1) * 128],
                        id_fr_sb,
                    )
                    oT = otT_pool.tile([128, 128], F32R, tag="oT")
                    nc.any.tensor_copy(out=oT, in_=pst)
                    oT_store[(it, cc)] = oT

            def emit_proj_ln(ih, onat):
                oT_store = {}
                for it in range(IT):
                    ia = ih * 512 + it * 128
                    emit_transposes(ih, it, onat, oT_store)
                    res = lnw.tile([128, D], F32, tag="res")
                    nc.sync.dma_start(res, qres.ap()[ia : ia + 128, :])
                    for dh in range(2):
                        ps_o = psS.tile([128, 512], F32, tag="pss", name="ps_o")
                        for cc in range(CC):
                            nc.tensor.matmul(
                                ps_o,
                                oT_store[(it, cc)],
                                pw_sb[:, cc, dh * 512 : (dh + 1) * 512],
                                start=(cc == 0),
                                stop=(cc == CC - 1),
                            )
                        nc.vector.tensor_add(
                            out=res[:, dh * 512 : (dh + 1) * 512],
                            in0=ps_o,
                            in1=res[:, dh * 512 : (dh + 1) * 512],
                        )
                    st = lnw.tile([128, 2, nc.vector.BN_STATS_DIM], F32, tag="bnst")
                    for g in range(2):
                        nc.vector.bn_stats(
                            out=st[:, g, :], in_=res[:, g * 512 : (g + 1) * 512]
                        )
                    mv = lnw.tile([128, nc.vector.BN_AGGR_DIM], F32, tag="bnag")
                    nc.vector.bn_aggr(out=mv, in_=st)
                    rstd = lnw.tile([128, 1], F32, tag="rstd")
                    nc.scalar.activation(
                        out=rstd, in_=mv[:, 1:2], func=AF.Sqrt, bias=eps_sb,
                        scale=1.0,
                    )
                    nc.vector.reciprocal(out=rstd, in_=rstd)
                    nc.vector.tensor_scalar(
                        out=res,
                        in0=res,
                        scalar1=mv[:, 0:1],
                        scalar2=rstd,
                        op0=ALU.subtract,
                        op1=ALU.mult,
                    )
                    nc.gpsimd.tensor_mul(out=res, in0=res, in1=g_sb)
                    nc.gpsimd.tensor_add(out=res, in0=res, in1=b_sb)
                    nc.sync.dma_start(ln.ap()[ia : ia + 128, :], res)

            # ---- pipelined emission: quarters unblock head groups (ih=0) ----
            onat0 = onat_pool.tile([128, IT, H * DK], F32R, tag="onat", name="onat0")
            load_xin("v", vT)
            emit_v_phase()
            load_xin("k", kT)
            load_xin("q", qT)
            for quarter in range(4):
                emit_kq_quarter(quarter)
                for h in range(4 * quarter, 4 * quarter + 4):
                    emit_head(0, h, onat0)
            p0ctx.close()
            lnw = ctx.enter_context(tc.tile_pool(name="lnw", bufs=2))
            pw_pool = ctx.enter_context(tc.tile_pool(name="pw", bufs=1))
            pw_sb = pw_pool.tile([128, CC, D], F32R)  # proj_w^T  4MiB
            nc.sync.dma_start(
                pw_sb, pwT.ap().rearrange("(cc p) d -> p cc d", p=128)
            )
            emit_proj_ln(0, onat0)
            load_masks(1)
            onat1 = onat_pool.tile([128, IT, H * DK], F32R, tag="onat", name="onat1")
            for h in range(H):
                emit_head(1, h, onat1)
            emit_proj_ln(1, onat1)

    _legalize_waits(nc)
    return nc


_NC = None


def _get_nc():
    global _NC
    if _NC is None:
        _NC = _build_nc()
    return _NC


def _prep_in_maps(inputs):
    q = np.asarray(inputs["q"], np.float32)
    k = np.asarray(inputs["k"], np.float32)
    v = np.asarray(inputs["v"], np.float32)
    mask = np.asarray(inputs["attn_mask"])
    w_q = np.asarray(inputs["w_q"], np.float32)
    w_k = np.asarray(inputs["w_k"], np.float32)
    w_v = np.asarray(inputs["w_v"], np.float32)
    proj_w = np.asarray(inputs["proj_w"], np.float32)
    proj_b = np.asarray(inputs["proj_b"], np.float32)
    ln_g = np.asarray(inputs["ln_g"], np.float32)
    ln_b = np.asarray(inputs["ln_b"], np.float32)

    wq_cat = np.ascontiguousarray(w_q.transpose(1, 0, 2).reshape(D, H * DK))
    wk_cat = np.ascontiguousarray(w_k.transpose(1, 0, 2).reshape(D, H * DK))
    wv_cat = np.ascontiguousarray(w_v.transpose(1, 0, 2).reshape(D, H * DK))
    pwT_h = np.ascontiguousarray(proj_w.T)
    gb_h = np.stack(
        [np.broadcast_to(ln_g, (128, D)), np.broadcast_to(ln_b, (128, D))]
    ).astype(ml_dtypes.bfloat16)
    id_bf_h = np.eye(128, dtype=ml_dtypes.bfloat16)
    id_fr_h = np.eye(128, dtype=np.float32)

    in_maps = []
    for b in range(B):
        m_b = mask[b]
        in_maps.append(
            {
                "qT": np.ascontiguousarray(q[b].T).astype(ml_dtypes.bfloat16),
                "kT": np.ascontiguousarray(k[b].T).astype(ml_dtypes.bfloat16),
                "vT": np.ascontiguousarray(v[b].T).astype(ml_dtypes.bfloat16),
                "wq": wq_cat.astype(ml_dtypes.bfloat16),
                "wk": wk_cat.astype(ml_dtypes.bfloat16),
                "wv": wv_cat.astype(ml_dtypes.bfloat16),
                "pwT": pwT_h,
                "qres": np.ascontiguousarray(q[b] + proj_b),
                "mbT": np.ascontiguousarray(
                    (NEG * m_b.T.astype(np.float32)).astype(ml_dtypes.bfloat16)
                ),
                "m01": np.ascontiguousarray(
                    (1.0 - m_b.astype(np.float32)).astype(ml_dtypes.bfloat16)
                ),
                "gb": gb_h,
                "id_bf": id_bf_h,
                "id_fr": id_fr_h,
            }
        )
    return in_maps


def kernel(**inputs):
    in_maps = _prep_in_maps(inputs)
    nc = _get_nc()
    res = run_bass_kernel_spmd(nc, in_maps, core_ids=list(range(B)))

    attn_all = np.stack([res.results[b]["attn"] for b in range(B)])  # [B, H, L, L]
    ln_all = np.stack([res.results[b]["ln"] for b in range(B)])      # [B, L, D]
    attn_flat = np.ascontiguousarray(
        attn_all.transpose(1, 0, 2, 3).reshape(H * B, L, L)
    )
    one_head = np.ascontiguousarray(attn_all[0, ::2])  # attn[2b, batch 0]
    return ln_all, attn_flat, one_head


# revision 39
# speedup vs baseline: 53.1077x; 1.1259x over previous
"""Trainium2 Bass kernel for nn_MultiHeadAttention_678604832830.

Strategy: data-parallel over batch B=8 -> one batch element per NeuronCore.
Each core computes, for its batch element b:
  - per-head attention probs  attn[h, i, j]  (written to HBM, f32)
  - ln = LayerNorm(attn_out @ proj_w.T + proj_b + q)   (written to HBM, f32)
Host reassembles attn_flat / one_head_attn from the per-core outputs.

Matmuls run as float32r (tf32-class precision at bf16 throughput); the
AV contraction runs in bf16 (attention probs are O(1), error ~1e-3).
Scores are computed twice: natural [i,j] for the attn output + softmax
sums (mask folded into one fused DVE scalar_tensor_tensor), and
transposed [j,i] for the AV contraction (mask added inside PSUM by an
identity (x) (-1e9*mask^T) matmul so exp() zeroes masked lanes free).

Projection phase runs V -> K -> Q so the head loop (which needs Q last)
starts as early as possible; all phases share two PSUM pools to avoid
bank-reuse serialization barriers.
"""

import sys

sys.path.insert(0, "/opt/trn_rl_repo")

import numpy as np
import ml_dtypes

import concourse.bass as bass
import concourse.mybir as mybir
import concourse.tile as tile
from concourse.bass_utils import run_bass_kernel_spmd

F32 = mybir.dt.float32
F32R = mybir.dt.float32r
BF16 = mybir.dt.bfloat16
AF = mybir.ActivationFunctionType
ALU = mybir.AluOpType

D, H, DK, B, L = 1024, 16, 64, 8, 1024
CC = 8   # chunks of 128 over the concat-head dim (H*DK = 1024)
DC = 8   # chunks of 128 over d_model
JC = 8   # chunks of 128 over key positions
IH = 2   # halves of the query dim (512 each)
IT = 4   # 128-row tiles per query half
SCALE = 1.0 / 32.0  # 1/sqrt(d_model)
NEG = -1.0e9
LN_EPS = 1e-6


def _patch_tile():
    """Lift the stale 192KB/partition SBUF cap to the usable 208KB."""
    if getattr(tile.TileContext, "_mha_patched", False):
        return
    try:
        from concourse import tile_utils

        if getattr(tile_utils, "max_sbuf_usage", 0) < 208 * 1024:
            tile_utils.max_sbuf_usage = 208 * 1024
    except Exception:
        pass
    tile.TileContext._mha_patched = True


def _legalize_waits(nc):
    """This walrus build accepts at most ONE sync wait per instruction (two
    for EventSemaphore). Tile sometimes attaches more (self-loading f32r
    matmuls have no LDWEIGHTS to spill onto; the final drain gets the whole
    global-clock set). Move excess waits onto same-engine nops inserted
    immediately before the offending instruction — the engine's in-order
    queue preserves the wait-before-execute semantics."""
    import bass_rust as _br

    n_split = 0
    for f in nc.m.functions:
        for bb in f.blocks:
            new = []
            changed = False
            for inst in bb.instructions:
                si = inst.sync_info
                cap = 2 if isinstance(inst, _br.InstEventSemaphore) else 1
                if si is not None and len(si.on_wait) > cap:
                    waits = list(si.on_wait)
                    SyncInfo = type(si)
                    for w in waits[:-cap]:
                        nop = _br.InstNoOp(
                            name=f"{inst.name}-lw{n_split}",
                            engine=inst.engine,
                            ins=[],
                            outs=[],
                        )
                        n_split += 1
                        nop.sync_info = SyncInfo(on_wait=[w], on_update=[])
                        nc.register_instruction(nop)
                        new.append(nop)
                    inst.sync_info = SyncInfo(
                        on_wait=waits[-cap:], on_update=list(si.on_update)
                    )
                    changed = True
                new.append(inst)
            if changed:
                bb.instructions = new


def _build_nc():
    _patch_tile()
    nc = bass.Bass()

    # ---- DRAM tensors (per-core; same NEFF on all 8 cores) ----
    qT = nc.dram_tensor("qT", [D, L], BF16, kind="ExternalInput")
    kT = nc.dram_tensor("kT", [D, L], BF16, kind="ExternalInput")
    vT = nc.dram_tensor("vT", [D, L], BF16, kind="ExternalInput")
    wq = nc.dram_tensor("wq", [D, H * DK], BF16, kind="ExternalInput")
    wk = nc.dram_tensor("wk", [D, H * DK], BF16, kind="ExternalInput")
    wv = nc.dram_tensor("wv", [D, H * DK], BF16, kind="ExternalInput")
    pwT = nc.dram_tensor("pwT", [H * DK, D], F32R, kind="ExternalInput")
    qres = nc.dram_tensor("qres", [L, D], F32, kind="ExternalInput")
    mbT = nc.dram_tensor("mbT", [L, L], BF16, kind="ExternalInput")   # -1e9*mask^T [j,i]
    m01 = nc.dram_tensor("m01", [L, L], BF16, kind="ExternalInput")   # 1-mask [i,j]
    gb = nc.dram_tensor("gb", [2, 128, D], BF16, kind="ExternalInput")  # ln_g/ln_b bcast
    id_bf = nc.dram_tensor("id_bf", [128, 128], BF16, kind="ExternalInput")
    id_fr = nc.dram_tensor("id_fr", [128, 128], F32R, kind="ExternalInput")
    attn = nc.dram_tensor("attn", [H, L, L], F32, kind="ExternalOutput")
    ln = nc.dram_tensor("ln", [L, D], F32, kind="ExternalOutput")

    with tile.TileContext(nc) as tc:
        from contextlib import ExitStack

        with ExitStack() as ctx:
            # ---------- persistent tiles (~13.1 MiB) ----------
            persist = ctx.enter_context(tc.tile_pool(name="persist", bufs=1))
            kt_sb = persist.tile([128, CC, L], F32R)      # K~^T  [c_in, cc, j]   4MiB
            qt_sb = persist.tile([128, CC, L], F32R)      # Q~^T  [c_in, cc, i]   4MiB
            v_sb = persist.tile([128, JC, H * DK], BF16)  # V~    [j_in, jc, call] 2MiB
            mbT_sb = persist.tile([128, JC, 512], BF16)   # mask bias^T half  1MiB
            m01_sb = persist.tile([128, IT, L], BF16)     # 0/1 mask rows half 1MiB
            id_bf_sb = persist.tile([128, 128], BF16)
            id_fr_sb = persist.tile([128, 128], F32R)
            g_sb = persist.tile([128, D], BF16)
            b_sb = persist.tile([128, D], BF16)
            eps_sb = persist.tile([128, 1], F32)
            nc.vector.memset(eps_sb, LN_EPS)
            nc.sync.dma_start(id_bf_sb, id_bf.ap())
            nc.sync.dma_start(id_fr_sb, id_fr.ap())
            nc.sync.dma_start(g_sb, gb.ap()[0])
            nc.sync.dma_start(b_sb, gb.ap()[1])

            # ---------- PSUM pools: 3x[128,1024] scores + 2x[128,512] ----
            psP = ctx.enter_context(tc.tile_pool(name="psP", bufs=3, space="PSUM"))
            psS = ctx.enter_context(tc.tile_pool(name="psS", bufs=2, space="PSUM"))

            eT_pool = ctx.enter_context(tc.tile_pool(name="eT", bufs=2))
            ework = ctx.enter_context(tc.tile_pool(name="ework", bufs=2))
            stats = ctx.enter_context(tc.tile_pool(name="stats", bufs=10))
            onat_pool = ctx.enter_context(tc.tile_pool(name="onat", bufs=1))
            otT_pool = ctx.enter_context(tc.tile_pool(name="otT", bufs=9))

            # ---------- phase P0 part 1: V~ ----------
            # ih=0 masks load first so heads can start during P0.
            def load_masks(ih):
                i0 = ih * 512
                nc.sync.dma_start(
                    mbT_sb,
                    mbT.ap()[:, i0 : i0 + 512].rearrange(
                        "(jc p) i -> p jc i", p=128
                    ),
                )
                nc.sync.dma_start(
                    m01_sb,
                    m01.ap()[i0 : i0 + 512, :].rearrange(
                        "(it p) j -> p it j", p=128
                    ),
                )

            load_masks(0)

            p0ctx = ExitStack()
            p0in = p0ctx.enter_context(tc.tile_pool(name="p0in", bufs=2))
            p0w = p0ctx.enter_context(tc.tile_pool(name="p0w", bufs=2))
            xins = {}

            def load_xin(kind, xdram):
                xin = p0in.tile([128, DC, L], BF16, tag="xin", name=f"xin_{kind}")
                xr = xdram.ap().rearrange("(dc p) l -> p dc l", p=128)
                for dc in range(DC):
                    nc.sync.dma_start(xin[:, dc, :], xr[:, dc, :])
                xins[kind] = xin

            def emit_v_phase():
                xin = xins["v"]
                wr = wv.ap().rearrange("(dc p) c -> p dc c", p=128)
                for quarter in range(4):
                    c0 = quarter * 256
                    win = p0w.tile([128, DC, 256], BF16, tag="winq",
                                   name="win_v")
                    for dc in range(DC):
                        nc.sync.dma_start(
                            win[:, dc, :], wr[:, dc, c0 : c0 + 256]
                        )
                    for lc in range(JC):
                        ps_full = psP.tile(
                            [128, L], F32, tag="ps", name="p0psv"
                        )
                        ps = ps_full[:, :256]
                        for dc in range(DC):
                            nc.tensor.matmul(
                                ps,
                                xin[:, dc, lc * 128 : (lc + 1) * 128],
                                win[:, dc, :],
                                start=(dc == 0),
                                stop=(dc == DC - 1),
                            )
                        nc.vector.tensor_copy(
                            out=v_sb[:, lc, c0 : c0 + 256], in_=ps
                        )

            def emit_kq_quarter(quarter):
                c0 = quarter * 256
                for kind, wdram in (("k", wk), ("q", wq)):
                    dst = kt_sb if kind == "k" else qt_sb
                    xin = xins[kind]
                    wr = wdram.ap().rearrange("(dc p) c -> p dc c", p=128)
                    win = p0w.tile([128, DC, 256], BF16, tag="winq",
                                   name=f"win_{kind}{quarter}")
                    for dc in range(DC):
                        nc.sync.dma_start(
                            win[:, dc, :], wr[:, dc, c0 : c0 + 256]
                        )
                    for ccl in range(2):
                        cc = quarter * 2 + ccl
                        ps = psP.tile([128, L], F32, tag="ps", name="p0ps")
                        for lh in range(2):
                            for dc in range(DC):
                                nc.tensor.matmul(
                                    ps[:, lh * 512 : (lh + 1) * 512],
                                    win[:, dc, ccl * 128 : (ccl + 1) * 128],
                                    xin[:, dc, lh * 512 : (lh + 1) * 512],
                                    start=(dc == 0),
                                    stop=(dc == DC - 1),
                                )
                        nc.vector.tensor_copy(out=dst[:, cc, :], in_=ps)

            def emit_head(ih, h, onat):
                i0 = ih * 512
                cc_h = h // 2
                cr = 64 * (h % 2)
                inv_s_tiles = []
                # pass A: natural scores -> attn rows + softmax sums
                for it in range(IT):
                    ia = i0 + it * 128
                    e_raw = ework.tile([128, L], BF16, tag="e_raw")
                    ps = psP.tile([128, L], F32, tag="ps", name="ps_a")
                    for jh in range(2):
                        nc.tensor.matmul(
                            ps[:, jh * 512 : (jh + 1) * 512],
                            qt_sb[cr : cr + 64, cc_h, ia : ia + 128],
                            kt_sb[
                                cr : cr + 64, cc_h, jh * 512 : (jh + 1) * 512
                            ],
                            start=True,
                            stop=True,
                        )
                    nc.scalar.activation(
                        out=e_raw, in_=ps, func=AF.Exp, scale=SCALE
                    )
                    e_m = ework.tile([128, L], BF16, tag="e_m")
                    s_sum = stats.tile([128, 1], F32, tag="s")
                    nc.vector.scalar_tensor_tensor(
                        out=e_m,
                        in0=e_raw,
                        scalar=1.0,
                        in1=m01_sb[:, it, :],
                        op0=ALU.mult,
                        op1=ALU.mult,
                        accum_out=s_sum,
                    )
                    inv_s = stats.tile([128, 1], F32, tag="inv")
                    nc.vector.reciprocal(out=inv_s, in_=s_sum)
                    inv_s_tiles.append(inv_s)
                    a_out = ework.tile([128, L], F32, tag="a_out")
                    nc.vector.tensor_scalar_mul(out=a_out, in0=e_m, scalar1=inv_s)
                    nc.sync.dma_start(attn.ap()[h, ia : ia + 128, :], a_out)

                # pass B: transposed scores (+mask in PSUM) -> e^T (bf16)
                eT = eT_pool.tile([128, JC, 512], BF16, tag="eT")
                for jp in range(JC // 2):
                    ps = psP.tile([128, L], F32, tag="ps", name="ps_b")
                    for sub in range(2):
                        jc = jp * 2 + sub
                        nc.tensor.matmul(
                            ps[:, sub * 512 : (sub + 1) * 512],
                            kt_sb[cr : cr + 64, cc_h, jc * 128 : (jc + 1) * 128],
                            qt_sb[cr : cr + 64, cc_h, i0 : i0 + 512],
                            start=True,
                            stop=False,
                        )
                        nc.tensor.matmul(
                            ps[:, sub * 512 : (sub + 1) * 512],
                            id_bf_sb,
                            mbT_sb[:, jc, :],
                            start=False,
                            stop=True,
                        )
                    nc.scalar.activation(
                        out=eT[:, jp * 2 : jp * 2 + 2, :],
                        in_=ps,
                        func=AF.Exp,
                        scale=SCALE,
                    )

                # AV: out[i, c] = sum_j eT[j,i] * V[j,c], normalized on evac
                for it in range(IT):
                    ps_full = psS.tile([128, 512], F32, tag="pss", name="ps_av")
                    ps = ps_full[:, :64]
                    for jc in range(JC):
                        nc.tensor.matmul(
                            ps,
                            eT[:, jc, it * 128 : (it + 1) * 128],
                            v_sb[:, jc, h * DK : (h + 1) * DK],
                            start=(jc == 0),
                            stop=(jc == JC - 1),
                        )
                    nc.vector.tensor_scalar_mul(
                        out=onat[:, it, h * DK : (h + 1) * DK],
                        in0=ps,
                        scalar1=inv_s_tiles[it],
                    )

            def emit_transposes(ih, it, onat, oT_store):
                for cc in range(CC):
                    pst_full = psS.tile(
                        [128, 512], F32R, tag="pss", name="ps_tr"
                    )
                    pst = pst_full[:, :128]
                    nc.tensor.transpose(
                        pst,
                        onat[:, it, cc * 128 : (cc + 1) * 128],
                        id_fr_sb,
                    )
                    oT = otT_pool.tile([128, 128], F32R, tag="oT")
                    nc.any.tensor_copy(out=oT, in_=pst)
                    oT_store[(it, cc)] = oT

            def emit_proj_ln(ih, onat):
                oT_store = {}
                for it in range(IT):
                    ia = ih * 512 + it * 128
                    emit_transposes(ih, it, onat, oT_store)
                    res = lnw.tile([128, D], F32, tag="res")
                    nc.sync.dma_start(res, qres.ap()[ia : ia + 128, :])
                    for dh in range(2):
                        ps_o = psS.tile([128, 512], F32, tag="pss", name="ps_o")
                        for cc in range(CC):
                            nc.tensor.matmul(
                                ps_o,
                                oT_store[(it, cc)],
                                pw_sb[:, cc, dh * 512 : (dh + 1) * 512],
                                start=(cc == 0),
                                stop=(cc == CC - 1),
                            )
                        nc.vector.tensor_add(
                            out=res[:, dh * 512 : (dh + 1) * 512],
                            in0=ps_o,
                            in1=res[:, dh * 512 : (dh + 1) * 512],
                        )
                    st = lnw.tile([128, 2, nc.vector.BN_STATS_DIM], F32, tag="bnst")
                    for g in range(2):
                        nc.vector.bn_stats(
                            out=st[:, g, :], in_=res[:, g * 512 : (g + 1) * 512]
                        )
                    mv = lnw.tile([128, nc.vector.BN_AGGR_DIM], F32, tag="bnag")
                    nc.vector.bn_aggr(out=mv, in_=st)
                    rstd = lnw.tile([128, 1], F32, tag="rstd")
                    nc.scalar.activation(
                        out=rstd, in_=mv[:, 1:2], func=AF.Sqrt, bias=eps_sb,
                        scale=1.0,
                    )
                    nc.vector.reciprocal(out=rstd, in_=rstd)
                    nc.vector.tensor_scalar(
                        out=res,
                        in0=res,
                        scalar1=mv[:, 0:1],
                        scalar2=rstd,
                        op0=ALU.subtract,
                        op1=ALU.mult,
                    )
                    nc.gpsimd.tensor_mul(out=res, in0=res, in1=g_sb)
                    nc.gpsimd.tensor_add(out=res, in0=res, in1=b_sb)
                    nc.sync.dma_start(ln.ap()[ia : ia + 128, :], res)

            # ---- pipelined emission: quarters unblock head groups (ih=0) ----
            onat0 = onat_pool.tile([128, IT, H * DK], F32R, tag="onat", name="onat0")
            load_xin("v", vT)
            emit_v_phase()
            load_xin("k", kT)
            load_xin("q", qT)
            for quarter in range(4):
                emit_kq_quarter(quarter)
                for h in range(4 * quarter, 4 * quarter + 4):
                    emit_head(0, h, onat0)
            p0ctx.close()
            lnw = ctx.enter_context(tc.tile_pool(name="lnw", bufs=2))
            pw_pool = ctx.enter_context(tc.tile_pool(name="pw", bufs=1))
            pw_sb = pw_pool.tile([128, CC, D], F32R)  # proj_w^T  4MiB
            nc.sync.dma_start(
                pw_sb, pwT.ap().rearrange("(cc p) d -> p cc d", p=128)
            )
            emit_proj_ln(0, onat0)
            load_masks(1)
            onat1 = onat_pool.tile([128, IT, H * DK], F32R, tag="onat", name="onat1")
            for h in range(H):
                emit_head(1, h, onat1)
            emit_proj_ln(1, onat1)

    _legalize_waits(nc)
    return nc


_NC = None


def _get_nc():
    global _NC
    if _NC is None:
        _NC = _build_nc()
    return _NC


def _prep_in_maps(inputs):
    q = np.asarray(inputs["q"], np.float32)
    k = np.asarray(inputs["k"], np.float32)
    v = np.asarray(inputs["v"], np.float32)
    mask = np.asarray(inputs["attn_mask"])
    w_q = np.asarray(inputs["w_q"], np.float32)
    w_k = np.asarray(inputs["w_k"], np.float32)
    w_v = np.asarray(inputs["w_v"], np.float32)
    proj_w = np.asarray(inputs["proj_w"], np.float32)
    proj_b = np.asarray(inputs["proj_b"], np.float32)
    ln_g = np.asarray(inputs["ln_g"], np.float32)
    ln_b = np.asarray(inputs["ln_b"], np.float32)

    wq_cat = np.ascontiguousarray(w_q.transpose(1, 0, 2).reshape(D, H * DK))
    wk_cat = np.ascontiguousarray(w_k.transpose(1, 0, 2).reshape(D, H * DK))
    wv_cat = np.ascontiguousarray(w_v.transpose(1, 0, 2).reshape(D, H * DK))
    pwT_h = np.ascontiguousarray(proj_w.T)
    gb_h = np.stack(
        [np.broadcast_to(ln_g, (128, D)), np.broadcast_to(ln_b, (128, D))]
    ).astype(ml_dtypes.bfloat16)
    id_bf_h = np.eye(128, dtype=ml_dtypes.bfloat16)
    id_fr_h = np.eye(128, dtype=np.float32)

    in_maps = []
    for b in range(B):
        m_b = mask[b]
        in_maps.append(
            {
                "qT": np.ascontiguousarray(q[b].T).astype(ml_dtypes.bfloat16),
                "kT": np.ascontiguousarray(k[b].T).astype(ml_dtypes.bfloat16),
                "vT": np.ascontiguousarray(v[b].T).astype(ml_dtypes.bfloat16),
                "wq": wq_cat.astype(ml_dtypes.bfloat16),
                "wk": wk_cat.astype(ml_dtypes.bfloat16),
                "wv": wv_cat.astype(ml_dtypes.bfloat16),
                "pwT": pwT_h,
                "qres": np.ascontiguousarray(q[b] + proj_b),
                "mbT": np.ascontiguousarray(
                    (NEG * m_b.T.astype(np.float32)).astype(ml_dtypes.bfloat16)
                ),
                "m01": np.ascontiguousarray(
                    (1.0 - m_b.astype(np.float32)).astype(ml_dtypes.bfloat16)
                ),
                "gb": gb_h,
                "id_bf": id_bf_h,
                "id_fr": id_fr_h,
            }
        )
    return in_maps


def kernel(**inputs):
    in_maps = _prep_in_maps(inputs)
    nc = _get_nc()
    res = run_bass_kernel_spmd(nc, in_maps, core_ids=list(range(B)))

    attn_all = np.stack([res.results[b]["attn"] for b in range(B)])  # [B, H, L, L]
    ln_all = np.stack([res.results[b]["ln"] for b in range(B)])      # [B, L, D]
    attn_flat = np.ascontiguousarray(
        attn_all.transpose(1, 0, 2, 3).reshape(H * B, L, L)
    )
    one_head = np.ascontiguousarray(attn_all[0, ::2])  # attn[2b, batch 0]
    return ln_all, attn_flat, one_head


# revision 42
# speedup vs baseline: 55.3887x; 1.0430x over previous
"""Trainium2 Bass kernel for nn_MultiHeadAttention_678604832830.

Strategy: data-parallel over batch B=8 -> one batch element per NeuronCore.
Each core computes, for its batch element b:
  - per-head attention probs  attn[h, i, j]  (written to HBM, f32)
  - ln = LayerNorm(attn_out @ proj_w.T + proj_b + q)   (written to HBM, f32)
Host reassembles attn_flat / one_head_attn from the per-core outputs.

Matmuls run as float32r (tf32-class precision at bf16 throughput); the
AV contraction runs in bf16 (attention probs are O(1), error ~1e-3).
Scores are computed twice: natural [i,j] for the attn output + softmax
sums (mask folded into one fused DVE scalar_tensor_tensor), and
transposed [j,i] for the AV contraction (mask added inside PSUM by an
identity (x) (-1e9*mask^T) matmul so exp() zeroes masked lanes free).

Projection phase runs V -> K -> Q so the head loop (which needs Q last)
starts as early as possible; all phases share two PSUM pools to avoid
bank-reuse serialization barriers.
"""

import sys

sys.path.insert(0, "/opt/trn_rl_repo")

import numpy as np
import ml_dtypes

import concourse.bass as bass
import concourse.mybir as mybir
import concourse.tile as tile
from concourse.bass_utils import run_bass_kernel_spmd

F32 = mybir.dt.float32
F32R = mybir.dt.float32r
BF16 = mybir.dt.bfloat16
AF = mybir.ActivationFunctionType
ALU = mybir.AluOpType

D, H, DK, B, L = 1024, 16, 64, 8, 1024
CC = 8   # chunks of 128 over the concat-head dim (H*DK = 1024)
DC = 8   # chunks of 128 over d_model
JC = 8   # chunks of 128 over key positions
IH = 2   # halves of the query dim (512 each)
IT = 4   # 128-row tiles per query half
SCALE = 1.0 / 32.0  # 1/sqrt(d_model)
NEG = -1.0e9
LN_EPS = 1e-6


def _patch_tile():
    """Lift the stale 192KB/partition SBUF cap to the usable 208KB."""
    if getattr(tile.TileContext, "_mha_patched", False):
        return
    try:
        from concourse import tile_utils

        if getattr(tile_utils, "max_sbuf_usage", 0) < 208 * 1024:
            tile_utils.max_sbuf_usage = 208 * 1024
    except Exception:
        pass
    tile.TileContext._mha_patched = True


def _legalize_waits(nc):
    """This walrus build accepts at most ONE sync wait per instruction (two
    for EventSemaphore). Tile sometimes attaches more (self-loading f32r
    matmuls have no LDWEIGHTS to spill onto; the final drain gets the whole
    global-clock set). Move excess waits onto same-engine nops inserted
    immediately before the offending instruction — the engine's in-order
    queue preserves the wait-before-execute semantics."""
    import bass_rust as _br

    n_split = 0
    for f in nc.m.functions:
        for bb in f.blocks:
            new = []
            changed = False
            for inst in bb.instructions:
                si = inst.sync_info
                cap = 2 if isinstance(inst, _br.InstEventSemaphore) else 1
                if si is not None and len(si.on_wait) > cap:
                    waits = list(si.on_wait)
                    SyncInfo = type(si)
                    for w in waits[:-cap]:
                        nop = _br.InstNoOp(
                            name=f"{inst.name}-lw{n_split}",
                            engine=inst.engine,
                            ins=[],
                            outs=[],
                        )
                        n_split += 1
                        nop.sync_info = SyncInfo(on_wait=[w], on_update=[])
                        nc.register_instruction(nop)
                        new.append(nop)
                    inst.sync_info = SyncInfo(
                        on_wait=waits[-cap:], on_update=list(si.on_update)
                    )
                    changed = True
                new.append(inst)
            if changed:
                bb.instructions = new


def _build_nc():
    _patch_tile()
    nc = bass.Bass()

    # ---- DRAM tensors (per-core; same NEFF on all 8 cores) ----
    qT = nc.dram_tensor("qT", [D, L], BF16, kind="ExternalInput")
    kT = nc.dram_tensor("kT", [D, L], BF16, kind="ExternalInput")
    vT = nc.dram_tensor("vT", [D, L], BF16, kind="ExternalInput")
    wq = nc.dram_tensor("wq", [D, H * DK], BF16, kind="ExternalInput")
    wk = nc.dram_tensor("wk", [D, H * DK], BF16, kind="ExternalInput")
    wv = nc.dram_tensor("wv", [D, H * DK], BF16, kind="ExternalInput")
    pwT = nc.dram_tensor("pwT", [H * DK, D], F32R, kind="ExternalInput")
    qres = nc.dram_tensor("qres", [L, D], F32, kind="ExternalInput")
    mbT = nc.dram_tensor("mbT", [L, L], BF16, kind="ExternalInput")   # -1e9*mask^T [j,i]
    m01 = nc.dram_tensor("m01", [L, L], BF16, kind="ExternalInput")   # 1-mask [i,j]
    gb = nc.dram_tensor("gb", [2, 128, D], BF16, kind="ExternalInput")  # ln_g/ln_b bcast
    id_bf = nc.dram_tensor("id_bf", [128, 128], BF16, kind="ExternalInput")
    id_fr = nc.dram_tensor("id_fr", [128, 128], F32R, kind="ExternalInput")
    attn = nc.dram_tensor("attn", [H, L, L], F32, kind="ExternalOutput")
    ln = nc.dram_tensor("ln", [L, D], F32, kind="ExternalOutput")

    with tile.TileContext(nc) as tc:
        from contextlib import ExitStack

        with ExitStack() as ctx:
            # ---------- persistent tiles (~13.1 MiB) ----------
            persist = ctx.enter_context(tc.tile_pool(name="persist", bufs=1))
            kt_sb = persist.tile([128, CC, L], F32R)      # K~^T  [c_in, cc, j]   4MiB
            qt_sb = persist.tile([128, CC, L], F32R)      # Q~^T  [c_in, cc, i]   4MiB
            v_sb = persist.tile([128, JC, H * DK], BF16)  # V~    [j_in, jc, call] 2MiB
            mbT_sb = persist.tile([128, JC, 512], BF16)   # mask bias^T half  1MiB
            m01_sb = persist.tile([128, IT, L], BF16)     # 0/1 mask rows half 1MiB
            id_bf_sb = persist.tile([128, 128], BF16)
            id_fr_sb = persist.tile([128, 128], F32R)
            g_sb = persist.tile([128, D], BF16)
            b_sb = persist.tile([128, D], BF16)
            eps_sb = persist.tile([128, 1], F32)
            nc.vector.memset(eps_sb, LN_EPS)
            nc.sync.dma_start(id_bf_sb, id_bf.ap())
            nc.sync.dma_start(id_fr_sb, id_fr.ap())
            nc.sync.dma_start(g_sb, gb.ap()[0])
            nc.sync.dma_start(b_sb, gb.ap()[1])

            # ---------- PSUM pools: 3x[128,1024] scores + 2x[128,512] ----
            psP = ctx.enter_context(tc.tile_pool(name="psP", bufs=3, space="PSUM"))
            psS = ctx.enter_context(tc.tile_pool(name="psS", bufs=2, space="PSUM"))

            eT_pool = ctx.enter_context(tc.tile_pool(name="eT", bufs=2))
            ework = ctx.enter_context(tc.tile_pool(name="ework", bufs=3))
            stats = ctx.enter_context(tc.tile_pool(name="stats", bufs=10))
            onat_pool = ctx.enter_context(tc.tile_pool(name="onat", bufs=1))
            otT_pool = ctx.enter_context(tc.tile_pool(name="otT", bufs=9))

            # ---------- phase P0 part 1: V~ ----------
            # ih=0 masks load first so heads can start during P0.
            def load_masks(ih):
                i0 = ih * 512
                nc.sync.dma_start(
                    mbT_sb,
                    mbT.ap()[:, i0 : i0 + 512].rearrange(
                        "(jc p) i -> p jc i", p=128
                    ),
                )
                nc.sync.dma_start(
                    m01_sb,
                    m01.ap()[i0 : i0 + 512, :].rearrange(
                        "(it p) j -> p it j", p=128
                    ),
                )

            load_masks(0)

            p0ctx = ExitStack()
            p0in = p0ctx.enter_context(tc.tile_pool(name="p0in", bufs=2))
            p0w = p0ctx.enter_context(tc.tile_pool(name="p0w", bufs=2))
            xins = {}

            def load_xin(kind, xdram):
                xin = p0in.tile([128, DC, L], BF16, tag="xin", name=f"xin_{kind}")
                xr = xdram.ap().rearrange("(dc p) l -> p dc l", p=128)
                for dc in range(DC):
                    nc.sync.dma_start(xin[:, dc, :], xr[:, dc, :])
                xins[kind] = xin

            def emit_v_phase():
                xin = xins["v"]
                wr = wv.ap().rearrange("(dc p) c -> p dc c", p=128)
                for quarter in range(4):
                    c0 = quarter * 256
                    win = p0w.tile([128, DC, 256], BF16, tag="winq",
                                   name="win_v")
                    for dc in range(DC):
                        nc.sync.dma_start(
                            win[:, dc, :], wr[:, dc, c0 : c0 + 256]
                        )
                    for lc in range(JC):
                        ps_full = psP.tile(
                            [128, L], F32, tag="ps", name="p0psv"
                        )
                        ps = ps_full[:, :256]
                        for dc in range(DC):
                            nc.tensor.matmul(
                                ps,
                                xin[:, dc, lc * 128 : (lc + 1) * 128],
                                win[:, dc, :],
                                start=(dc == 0),
                                stop=(dc == DC - 1),
                            )
                        nc.vector.tensor_copy(
                            out=v_sb[:, lc, c0 : c0 + 256], in_=ps
                        )

            def emit_kq_quarter(quarter):
                c0 = quarter * 256
                for kind, wdram in (("k", wk), ("q", wq)):
                    dst = kt_sb if kind == "k" else qt_sb
                    xin = xins[kind]
                    wr = wdram.ap().rearrange("(dc p) c -> p dc c", p=128)
                    win = p0w.tile([128, DC, 256], BF16, tag="winq",
                                   name=f"win_{kind}{quarter}")
                    for dc in range(DC):
                        nc.sync.dma_start(
                            win[:, dc, :], wr[:, dc, c0 : c0 + 256]
                        )
                    for ccl in range(2):
                        cc = quarter * 2 + ccl
                        ps = psP.tile([128, L], F32, tag="ps", name="p0ps")
                        for lh in range(2):
                            for dc in range(DC):
                                nc.tensor.matmul(
                                    ps[:, lh * 512 : (lh + 1) * 512],
                                    win[:, dc, ccl * 128 : (ccl + 1) * 128],
                                    xin[:, dc, lh * 512 : (lh + 1) * 512],
                                    start=(dc == 0),
                                    stop=(dc == DC - 1),
                                )
                        nc.vector.tensor_copy(out=dst[:, cc, :], in_=ps)

            def emit_head(ih, h, onat):
                i0 = ih * 512
                cc_h = h // 2
                cr = 64 * (h % 2)
                inv_s_tiles = []
                # pass A: natural scores -> attn rows + softmax sums
                for it in range(IT):
                    ia = i0 + it * 128
                    e_raw = ework.tile([128, L], BF16, tag="e_raw")
                    ps = psP.tile([128, L], F32, tag="ps", name="ps_a")
                    for jh in range(2):
                        nc.tensor.matmul(
                            ps[:, jh * 512 : (jh + 1) * 512],
                            qt_sb[cr : cr + 64, cc_h, ia : ia + 128],
                            kt_sb[
                                cr : cr + 64, cc_h, jh * 512 : (jh + 1) * 512
                            ],
                            start=True,
                            stop=True,
                        )
                    nc.scalar.activation(
                        out=e_raw, in_=ps, func=AF.Exp, scale=SCALE
                    )
                    e_m = ework.tile([128, L], BF16, tag="e_m")
                    s_sum = stats.tile([128, 1], F32, tag="s")
                    nc.vector.scalar_tensor_tensor(
                        out=e_m,
                        in0=e_raw,
                        scalar=1.0,
                        in1=m01_sb[:, it, :],
                        op0=ALU.mult,
                        op1=ALU.mult,
                        accum_out=s_sum,
                    )
                    inv_s = stats.tile([128, 1], F32, tag="inv")
                    nc.vector.reciprocal(out=inv_s, in_=s_sum)
                    inv_s_tiles.append(inv_s)
                    a_out = ework.tile([128, L], F32, tag="a_out")
                    nc.vector.tensor_scalar_mul(out=a_out, in0=e_m, scalar1=inv_s)
                    nc.sync.dma_start(attn.ap()[h, ia : ia + 128, :], a_out)

                # pass B: transposed scores (+mask in PSUM) -> e^T (bf16)
                eT = eT_pool.tile([128, JC, 512], BF16, tag="eT")
                for jp in range(JC // 2):
                    ps = psP.tile([128, L], F32, tag="ps", name="ps_b")
                    for sub in range(2):
                        jc = jp * 2 + sub
                        nc.tensor.matmul(
                            ps[:, sub * 512 : (sub + 1) * 512],
                            kt_sb[cr : cr + 64, cc_h, jc * 128 : (jc + 1) * 128],
                            qt_sb[cr : cr + 64, cc_h, i0 : i0 + 512],
                            start=True,
                            stop=False,
                        )
                        nc.tensor.matmul(
                            ps[:, sub * 512 : (sub + 1) * 512],
                            id_bf_sb,
                            mbT_sb[:, jc, :],
                            start=False,
                            stop=True,
                        )
                    nc.scalar.activation(
                        out=eT[:, jp * 2 : jp * 2 + 2, :],
                        in_=ps,
                        func=AF.Exp,
                        scale=SCALE,
                    )

                # AV: out[i, c] = sum_j eT[j,i] * V[j,c], normalized on evac
                for it in range(IT):
                    ps_full = psS.tile([128, 512], F32, tag="pss", name="ps_av")
                    ps = ps_full[:, :64]
                    for jc in range(JC):
                        nc.tensor.matmul(
                            ps,
                            eT[:, jc, it * 128 : (it + 1) * 128],
                            v_sb[:, jc, h * DK : (h + 1) * DK],
                            start=(jc == 0),
                            stop=(jc == JC - 1),
                        )
                    nc.vector.tensor_scalar_mul(
                        out=onat[:, it, h * DK : (h + 1) * DK],
                        in0=ps,
                        scalar1=inv_s_tiles[it],
                    )

            def emit_transposes(ih, it, onat, oT_store):
                for cc in range(CC):
                    pst_full = psS.tile(
                        [128, 512], F32R, tag="pss", name="ps_tr"
                    )
                    pst = pst_full[:, :128]
                    nc.tensor.transpose(
                        pst,
                        onat[:, it, cc * 128 : (cc + 1) * 128],
                        id_fr_sb,
                    )
                    oT = otT_pool.tile([128, 128], F32R, tag="oT")
                    nc.any.tensor_copy(out=oT, in_=pst)
                    oT_store[(it, cc)] = oT

            def emit_proj_ln(ih, onat):
                oT_store = {}
                for it in range(IT):
                    ia = ih * 512 + it * 128
                    emit_transposes(ih, it, onat, oT_store)
                    res = lnw.tile([128, D], F32, tag="res")
                    nc.sync.dma_start(res, qres.ap()[ia : ia + 128, :])
                    for dh in range(2):
                        ps_o = psS.tile([128, 512], F32, tag="pss", name="ps_o")
                        for cc in range(CC):
                            nc.tensor.matmul(
                                ps_o,
                                oT_store[(it, cc)],
                                pw_sb[:, cc, dh * 512 : (dh + 1) * 512],
                                start=(cc == 0),
                                stop=(cc == CC - 1),
                            )
                        nc.vector.tensor_add(
                            out=res[:, dh * 512 : (dh + 1) * 512],
                            in0=ps_o,
                            in1=res[:, dh * 512 : (dh + 1) * 512],
                        )
                    st = lnw.tile([128, 2, nc.vector.BN_STATS_DIM], F32, tag="bnst")
                    for g in range(2):
                        nc.vector.bn_stats(
                            out=st[:, g, :], in_=res[:, g * 512 : (g + 1) * 512]
                        )
                    mv = lnw.tile([128, nc.vector.BN_AGGR_DIM], F32, tag="bnag")
                    nc.vector.bn_aggr(out=mv, in_=st)
                    rstd = lnw.tile([128, 1], F32, tag="rstd")
                    nc.scalar.activation(
                        out=rstd, in_=mv[:, 1:2], func=AF.Sqrt, bias=eps_sb,
                        scale=1.0,
                    )
                    nc.vector.reciprocal(out=rstd, in_=rstd)
                    nc.vector.tensor_scalar(
                        out=res,
                        in0=res,
                        scalar1=mv[:, 0:1],
                        scalar2=rstd,
                        op0=ALU.subtract,
                        op1=ALU.mult,
                    )
                    nc.gpsimd.tensor_mul(out=res, in0=res, in1=g_sb)
                    nc.gpsimd.tensor_add(out=res, in0=res, in1=b_sb)
                    nc.sync.dma_start(ln.ap()[ia : ia + 128, :], res)

            # ---- pipelined emission: quarters unblock head groups (ih=0) ----
            onat0 = onat_pool.tile([128, IT, H * DK], F32R, tag="onat", name="onat0")
            load_xin("v", vT)
            emit_v_phase()
            load_xin("k", kT)
            load_xin("q", qT)
            for quarter in range(4):
                emit_kq_quarter(quarter)
                for h in range(4 * quarter, 4 * quarter + 4):
                    emit_head(0, h, onat0)
            p0ctx.close()
            lnw = ctx.enter_context(tc.tile_pool(name="lnw", bufs=2))
            pw_pool = ctx.enter_context(tc.tile_pool(name="pw", bufs=1))
            pw_sb = pw_pool.tile([128, CC, D], F32R)  # proj_w^T  4MiB
            nc.sync.dma_start(
                pw_sb, pwT.ap().rearrange("(cc p) d -> p cc d", p=128)
            )
            emit_proj_ln(0, onat0)
            load_masks(1)
            onat1 = onat_pool.tile([128, IT, H * DK], F32R, tag="onat", name="onat1")
            for h in range(H):
                emit_head(1, h, onat1)
            emit_proj_ln(1, onat1)

    _legalize_waits(nc)
    return nc


_NC = None


def _get_nc():
    global _NC
    if _NC is None:
        _NC = _build_nc()
    return _NC


def _prep_in_maps(inputs):
    q = np.asarray(inputs["q"], np.float32)
    k = np.asarray(inputs["k"], np.float32)
    v = np.asarray(inputs["v"], np.float32)
    mask = np.asarray(inputs["attn_mask"])
    w_q = np.asarray(inputs["w_q"], np.float32)
    w_k = np.asarray(inputs["w_k"], np.float32)
    w_v = np.asarray(inputs["w_v"], np.float32)
    proj_w = np.asarray(inputs["proj_w"], np.float32)
    proj_b = np.asarray(inputs["proj_b"], np.float32)
    ln_g = np.asarray(inputs["ln_g"], np.float32)
    ln_b = np.asarray(inputs["ln_b"], np.float32)

    wq_cat = np.ascontiguousarray(w_q.transpose(1, 0, 2).reshape(D, H * DK))
    wk_cat = np.ascontiguousarray(w_k.transpose(1, 0, 2).reshape(D, H * DK))
    wv_cat = np.ascontiguousarray(w_v.transpose(1, 0, 2).reshape(D, H * DK))
    pwT_h = np.ascontiguousarray(proj_w.T)
    gb_h = np.stack(
        [np.broadcast_to(ln_g, (128, D)), np.broadcast_to(ln_b, (128, D))]
    ).astype(ml_dtypes.bfloat16)
    id_bf_h = np.eye(128, dtype=ml_dtypes.bfloat16)
    id_fr_h = np.eye(128, dtype=np.float32)

    in_maps = []
    for b in range(B):
        m_b = mask[b]
        in_maps.append(
            {
                "qT": np.ascontiguousarray(q[b].T).astype(ml_dtypes.bfloat16),
                "kT": np.ascontiguousarray(k[b].T).astype(ml_dtypes.bfloat16),
                "vT": np.ascontiguousarray(v[b].T).astype(ml_dtypes.bfloat16),
                "wq": wq_cat.astype(ml_dtypes.bfloat16),
                "wk": wk_cat.astype(ml_dtypes.bfloat16),
                "wv": wv_cat.astype(ml_dtypes.bfloat16),
                "pwT": pwT_h,
                "qres": np.ascontiguousarray(q[b] + proj_b),
                "mbT": np.ascontiguousarray(
                    (NEG * m_b.T.astype(np.float32)).astype(ml_dtypes.bfloat16)
                ),
                "m01": np.ascontiguousarray(
                    (1.0 - m_b.astype(np.float32)).astype(ml_dtypes.bfloat16)
                ),
                "gb": gb_h,
                "id_bf": id_bf_h,
                "id_fr": id_fr_h,
            }
        )
    return in_maps


def kernel(**inputs):
    in_maps = _prep_in_maps(inputs)
    nc = _get_nc()
    res = run_bass_kernel_spmd(nc, in_maps, core_ids=list(range(B)))

    attn_all = np.stack([res.results[b]["attn"] for b in range(B)])  # [B, H, L, L]
    ln_all = np.stack([res.results[b]["ln"] for b in range(B)])      # [B, L, D]
    attn_flat = np.ascontiguousarray(
        attn_all.transpose(1, 0, 2, 3).reshape(H * B, L, L)
    )
    one_head = np.ascontiguousarray(attn_all[0, ::2])  # attn[2b, batch 0]
    return ln_all, attn_flat, one_head
